# revision 1
# baseline (speedup 1.0000x reference)
"""DLRM forward (bottom MLP + 26-table EmbeddingBag + dot interaction + top MLP)
on 8 Trainium2 NeuronCores via Bass/Tile.

Sharding: batch-parallel. Each core handles 1024 of the 8192 samples and owns a
replicated copy of all 26 embedding tables in its HBM. No collectives.

Gather strategy: `dma_gather` (SWDGE) needs int16 row indices, so each 200000-row
table is addressed as 7 chunks of <=32768 rows. Per (table, chunk) the host
routes that chunk's indices into a compacted int16 stream (bag-sorted order is
preserved). Pooling of the gathered rows into per-bag sums runs on the
TensorEngine: for each 128-row gathered tile, a one-hot assignment matrix A
(built on-chip from a host-supplied relative-bag vector via is_equal against an
iota row) maps rows -> bags of one 128-bag window, accumulating in PSUM.

The SPMD program must be identical across cores, so tile counts per
(table, chunk) are padded to the max over the 8 cores and each tile emits
pooling matmuls for the union of bag-windows any core touches there; a core's
`bagrel` data zeroes the windows it does not use.
"""

import os
import sys
import time

import numpy as np

# ---------------------------------------------------------------- constants
B = 8192
L = 10
NT = 26
V = 200000
D = 64
NCORE = 8
BC = B // NCORE            # 1024 samples per core
SLOT = BC * L              # 10240 gathers per (core, table)
SCH = 32768                # chunk rows (int16-addressable)
NCH = (V + SCH - 1) // SCH  # 7
TILE = 128
TP = 16                    # max tiles per gather piece (ring slot)
JB = 16                    # A-matrix build batch (jobs)
NWIN = BC // TILE          # 8 bag windows per core
PAD_BAGREL = -512.0


# ---------------------------------------------------------------- host prep
def _prep(lS_i):
    """Compute the shared static structure + per-core device arrays."""
    lS = np.asarray(lS_i).astype(np.int64)

    seg_loc = {}
    seg_bag = {}
    nrows = np.zeros((NCORE, NT, NCH), np.int64)
    bag_of_pos = np.arange(SLOT, dtype=np.int64) // L
    for c in range(NCORE):
        for t in range(NT):
            idx = lS[t, c * SLOT:(c + 1) * SLOT].astype(np.int64)
            ch = idx >> 15
            order = np.argsort(ch, kind="stable")
            sidx = idx[order]
            sbag = bag_of_pos[order]
            sch = ch[order]
            bounds = np.searchsorted(sch, np.arange(NCH + 1))
            for k in range(NCH):
                lo, hi = bounds[k], bounds[k + 1]
                seg_loc[(c, t, k)] = (sidx[lo:hi] - (k << 15)).astype(np.int16)
                seg_bag[(c, t, k)] = sbag[lo:hi]
                nrows[c, t, k] = hi - lo

    T_tk = np.maximum(1, ((nrows.max(axis=0) + TILE - 1) // TILE)).astype(np.int64)

    # static walk: gather pieces + pooling jobs
    pieces = []        # (t, k, ntiles, idxcol0)
    piece_tile0 = []   # first tile index of the piece within its (t, k)
    jobs = []          # [t, w, piece_idx, tile_in_piece, start, stop]
    idxcols = 0
    first_last = {}
    for t in range(NT):
        for k in range(NCH):
            ntk = int(T_tk[t, k])
            tile0 = 0
            while tile0 < ntk:
                nt_p = min(TP, ntk - tile0)
                p_idx = len(pieces)
                pieces.append((t, k, nt_p, idxcols))
                piece_tile0.append(tile0)
                for i in range(nt_p):
                    gt = tile0 + i
                    wins = set()
                    for c in range(NCORE):
                        bags = seg_bag[(c, t, k)][gt * TILE:(gt + 1) * TILE]
                        if len(bags):
                            wins.update(np.unique(bags // TILE).tolist())
                    if not wins:
                        wins = {0}
                    for w in sorted(wins):
                        j = len(jobs)
                        jobs.append([t, int(w), p_idx, i, False, False])
                        if (t, w) not in first_last:
                            first_last[(t, w)] = [j, j]
                        else:
                            first_last[(t, w)][1] = j
                idxcols += nt_p * (TILE // 16)
                tile0 += nt_p
    for t in range(NT):
        for w in range(NWIN):
            assert (t, w) in first_last, (t, w)
    for (t, w), (f, l_) in first_last.items():
        jobs[f][4] = True
        jobs[l_][5] = True
    njobs = len(jobs)

    idx16 = np.zeros((NCORE, 16, idxcols), np.int16)
    bagrel = np.full((NCORE, TILE, njobs), PAD_BAGREL, np.float16)
    for c in range(NCORE):
        for p_idx, (t, k, nt_p, col0) in enumerate(pieces):
            tile0 = piece_tile0[p_idx]
            loc = seg_loc[(c, t, k)]
            n = len(loc)
            lo = tile0 * TILE
            hi = min(n, (tile0 + nt_p) * TILE)
            chunk = np.zeros(nt_p * TILE, np.int16)
            if hi > lo:
                chunk[: hi - lo] = loc[lo:hi]
            # element i -> [i % 16, i // 16]
            idx16[c, :, col0:col0 + nt_p * (TILE // 16)] = chunk.reshape(-1, 16).T

        for j, (t, w, p_idx, i, _s, _e) in enumerate(jobs):
            _t, k, nt_p, col0 = pieces[p_idx]
            gt = piece_tile0[p_idx] + i
            seg = seg_bag[(c, t, k)][gt * TILE:(gt + 1) * TILE]
            if len(seg):
                bagrel[c, : len(seg), j] = (
                    seg.astype(np.float32) - 128.0 * w).astype(np.float16)

    idx16_rep = np.tile(idx16, (1, 8, 1))      # replicate into 8 bands of 16
    static = dict(pieces=pieces, piece_tile0=piece_tile0, jobs=jobs,
                  idxcols=idxcols, njobs=njobs, T_tk=T_tk)
    return static, idx16_rep, bagrel


# ---------------------------------------------------------------- program
def _build(static):
    import concourse.tile as tile
    from concourse import bacc, mybir
    from concourse.masks import make_identity

    f32 = mybir.dt.float32
    bf16 = mybir.dt.bfloat16
    fp16 = mybir.dt.float16
    i16 = mybir.dt.int16
    AF = mybir.ActivationFunctionType
    ALU = mybir.AluOpType
    AX = mybir.AxisListType

    pieces = static["pieces"]
    jobs = static["jobs"]
    idxcols = static["idxcols"]
    njobs = static["njobs"]

    nc = bacc.Bacc("TRN2", target_bir_lowering=False, debug=False,
                   num_devices=NCORE)

    tab = nc.dram_tensor("tab", [NT, V, D], f32, kind="ExternalInput").ap()
    idx16_d = nc.dram_tensor("idx16", [128, idxcols], i16, kind="ExternalInput").ap()
    bagrel_d = nc.dram_tensor("bagrel", [128, njobs], fp16, kind="ExternalInput").ap()
    iota_d = nc.dram_tensor("iotaf", [128, 128], fp16, kind="ExternalInput").ap()
    xT_d = nc.dram_tensor("xT", [13, BC], f32, kind="ExternalInput").ap()
    w1t_d = nc.dram_tensor("w1t", [13, 512], f32, kind="ExternalInput").ap()
    w2t_d = nc.dram_tensor("w2t", [128, 4, 256], f32, kind="ExternalInput").ap()
    w3t_d = nc.dram_tensor("w3t", [128, 2, 64], f32, kind="ExternalInput").ap()
    b1_d = nc.dram_tensor("b1", [128, 4], f32, kind="ExternalInput").ap()
    b2_d = nc.dram_tensor("b2", [128, 2], f32, kind="ExternalInput").ap()
    b3_d = nc.dram_tensor("b3", [64, 1], f32, kind="ExternalInput").ap()
    tw1t_d = nc.dram_tensor("tw1t", [128, 4, 512], f32, kind="ExternalInput").ap()
    tw2t_d = nc.dram_tensor("tw2t", [128, 4, 256], f32, kind="ExternalInput").ap()
    tw3t_d = nc.dram_tensor("tw3t", [128, 2, 1], f32, kind="ExternalInput").ap()
    tb1_d = nc.dram_tensor("tb1", [128, 4], f32, kind="ExternalInput").ap()
    tb2_d = nc.dram_tensor("tb2", [128, 2], f32, kind="ExternalInput").ap()
    tb3_d = nc.dram_tensor("tb3", [1, 1], f32, kind="ExternalInput").ap()
    out_d = nc.dram_tensor("out", [BC, 1], f32, kind="ExternalOutput").ap()

    idx16_s = nc.alloc_sbuf_tensor("idx16_s", [128, idxcols], i16).ap()
    bagrel_s = nc.alloc_sbuf_tensor("bagrel_s", [128, njobs], fp16).ap()
    iota_s = nc.alloc_sbuf_tensor("iota_s", [128, 128], fp16).ap()
    tall = nc.alloc_sbuf_tensor("tall", [128, NWIN, NT + 1, D], bf16).ap()
    r_all = nc.alloc_sbuf_tensor("r_all", [128, NWIN, 416], f32).ap()
    itmp = nc.alloc_sbuf_tensor("itmp", [128, NT, D], bf16).ap()
    xT_s = nc.alloc_sbuf_tensor("xT_s", [13, BC], f32).ap()
    w1t_s = nc.alloc_sbuf_tensor("w1t_s", [13, 512], f32).ap()
    w2t_s = nc.alloc_sbuf_tensor("w2t_s", [128, 4, 256], f32).ap()
    w3t_s = nc.alloc_sbuf_tensor("w3t_s", [128, 2, 64], f32).ap()
    b1_s = nc.alloc_sbuf_tensor("b1_s", [128, 4], f32).ap()
    b2_s = nc.alloc_sbuf_tensor("b2_s", [128, 2], f32).ap()
    b3_s = nc.alloc_sbuf_tensor("b3_s", [64, 1], f32).ap()
    tw1t_s = nc.alloc_sbuf_tensor("tw1t_s", [128, 4, 512], f32).ap()
    tw2t_s = nc.alloc_sbuf_tensor("tw2t_s", [128, 4, 256], f32).ap()
    tw3t_s = nc.alloc_sbuf_tensor("tw3t_s", [128, 2, 1], f32).ap()
    tb1_s = nc.alloc_sbuf_tensor("tb1_s", [128, 4], f32).ap()
    tb2_s = nc.alloc_sbuf_tensor("tb2_s", [128, 2], f32).ap()
    tb3_s = nc.alloc_sbuf_tensor("tb3_s", [1, 1], f32).ap()
    h1t = nc.alloc_sbuf_tensor("h1t", [128, 4, BC], f32).ap()
    h2t = nc.alloc_sbuf_tensor("h2t", [128, 2, BC], f32).ap()
    h3t = nc.alloc_sbuf_tensor("h3t", [64, BC], f32).ap()
    ident = nc.alloc_sbuf_tensor("ident", [128, 128], f32).ap()
    p_s = nc.alloc_sbuf_tensor("p_s", [1, BC], f32).ap()

    with tile.TileContext(nc) as tc:
        with (
            tc.tile_pool(name="g", bufs=6) as gp,
            tc.tile_pool(name="ab", bufs=3) as abp,
            tc.tile_pool(name="gbf", bufs=4) as gbfp,
            tc.tile_pool(name="pp", bufs=2, space="PSUM") as pp,
            tc.tile_pool(name="mp", bufs=2, space="PSUM") as mp,
            tc.tile_pool(name="tp", bufs=2, space="PSUM") as tpp,
        ):
            # ---- input loads
            nc.sync.dma_start(idx16_s, idx16_d)
            nc.sync.dma_start(bagrel_s, bagrel_d)
            nc.sync.dma_start(iota_s, iota_d)
            nc.sync.dma_start(xT_s, xT_d)
            for a, b_ in [(w1t_s, w1t_d), (w2t_s, w2t_d), (w3t_s, w3t_d),
                          (b1_s, b1_d), (b2_s, b2_d), (b3_s, b3_d),
                          (tw1t_s, tw1t_d), (tw2t_s, tw2t_d), (tw3t_s, tw3t_d),
                          (tb1_s, tb1_d), (tb2_s, tb2_d), (tb3_s, tb3_d)]:
                nc.sync.dma_start(a, b_)
            make_identity(nc, ident)
            nc.vector.memset(r_all, 0.0)
            nc.vector.memset(tall, 0.0)

            # ---- bottom MLP (no dependence on gathers; overlaps them)
            for m in range(4):
                for n2 in range(2):
                    ps = mp.tile([128, 512], f32, tag="mp")
                    nc.tensor.matmul(ps, w1t_s[:, m * 128:(m + 1) * 128],
                                     xT_s[:, n2 * 512:(n2 + 1) * 512],
                                     start=True, stop=True)
                    nc.scalar.activation(h1t[:, m, n2 * 512:(n2 + 1) * 512], ps,
                                         AF.Relu, bias=b1_s[:, m:m + 1])
            for m in range(2):
                for n2 in range(2):
                    ps = mp.tile([128, 512], f32, tag="mp")
                    for kk in range(4):
                        nc.tensor.matmul(ps, w2t_s[:, kk, m * 128:(m + 1) * 128],
                                         h1t[:, kk, n2 * 512:(n2 + 1) * 512],
                                         start=(kk == 0), stop=(kk == 3))
                    nc.scalar.activation(h2t[:, m, n2 * 512:(n2 + 1) * 512], ps,
                                         AF.Relu, bias=b2_s[:, m:m + 1])
            for n2 in range(2):
                ps = mp.tile([128, 512], f32, tag="mp")
                for kk in range(2):
                    nc.tensor.matmul(ps[0:64, :], w3t_s[:, kk, :],
                                     h2t[:, kk, n2 * 512:(n2 + 1) * 512],
                                     start=(kk == 0), stop=(kk == 1))
                nc.scalar.activation(h3t[:, n2 * 512:(n2 + 1) * 512], ps[0:64, :],
                                     AF.Relu, bias=b3_s)
            for blk in range(NWIN):
                ps = tpp.tile([128, 128], f32, tag="tp")
                nc.tensor.transpose(ps[:, 0:64],
                                    h3t[:, blk * 128:(blk + 1) * 128],
                                    ident[0:64, 0:64])
                nc.vector.tensor_copy(r_all[:, blk, 0:64], ps[:, 0:64])
                nc.scalar.activation(tall[:, blk, 0, :], ps[:, 0:64], AF.Copy)

            # ---- gather + pool
            np_lim = int(os.environ.get("DLRM_NPIECES", "1000000"))
            job_ptr = 0
            a_tile = None
            a_base = -1
            pooled = None
            cur_t = -1
            for p_idx, (t, k, nt_p, col0) in enumerate(pieces):
                if p_idx >= np_lim:
                    job_ptr = len(jobs)
                    break
                if t != cur_t:
                    if pooled is not None:
                        nc.scalar.activation(tall[:, :, cur_t + 1, :],
                                             pooled[:], AF.Copy)
                    pooled = pp.tile([128, NWIN, 64], f32, tag="pp")
                    cur_t = t
                rows = min(V - k * SCH, SCH)
                src = tab[t, k * SCH:k * SCH + rows, :]
                g = gp.tile([128, TP, 64], f32, tag="gbuf")
                n_idx = nt_p * TILE
                nc.gpsimd.dma_gather(
                    g[:, :nt_p, :], src,
                    idx16_s[:, col0:col0 + nt_p * 8],
                    num_idxs=n_idx, num_idxs_reg=n_idx,
                    elem_size=64, elem_step=64, single_packet=False,
                )
                gbf = gbfp.tile([128, TP, 64], bf16, tag="gbf")
                nc.scalar.activation(gbf[:, :nt_p, :], g[:, :nt_p, :], AF.Copy)
                while job_ptr < len(jobs) and jobs[job_ptr][2] == p_idx:
                    jt, w, _p, i, st, en = jobs[job_ptr]
                    if a_tile is None or job_ptr >= a_base + JB:
                        a_base = job_ptr
                        nb = min(JB, njobs - a_base)
                        a_tile = abp.tile([128, JB, 128], bf16, tag="ab")
                        nc.vector.tensor_tensor(
                            a_tile[:, :nb, :],
                            bagrel_s[:, a_base:a_base + nb, None]
                            .to_broadcast([128, nb, 128]),
                            iota_s[:, None, :].to_broadcast([128, nb, 128]),
                            op=ALU.is_equal,
                        )
                    nc.tensor.matmul(pooled[:, w, :],
                                     a_tile[:, job_ptr - a_base, :],
                                     gbf[:, i, :],
                                     start=st, stop=en, skip_group_check=True)
                    job_ptr += 1
            assert job_ptr == len(jobs)
            if pooled is not None:
                nc.scalar.activation(tall[:, :, cur_t + 1, :], pooled[:], AF.Copy)

            # ---- dot interaction (Z lower triangle -> r_all cols 64..415)
            off = 0
            for n in range(1, NT + 1):
                for blk in range(NWIN):
                    nc.vector.tensor_tensor(
                        itmp[:, 0:n, :],
                        tall[:, blk, n, None, :].to_broadcast([128, n, 64]),
                        tall[:, blk, 0:n, :],
                        op=ALU.mult,
                    )
                    nc.vector.tensor_reduce(
                        r_all[:, blk, 64 + off:64 + off + n],
                        itmp[:, 0:n, :], axis=AX.X, op=ALU.add,
                    )
                off += n

            # ---- top MLP (R^T tiles share the gather pool slots)
            rt = []
            for f in range(4):
                rt.append(gp.tile([128, 1024], f32, tag="gbuf", name=f"rt{f}"))
            nc.vector.memset(rt[3][:, :], 0.0)
            for f in range(4):
                wdt = 128 if f < 3 else 32
                for blk in range(NWIN):
                    ps = tpp.tile([128, 128], f32, tag="tp")
                    nc.tensor.transpose(ps[0:wdt, :],
                                        r_all[:, blk, f * 128:f * 128 + wdt],
                                        ident)
                    nc.vector.tensor_copy(
                        rt[f][0:wdt, blk * 128:(blk + 1) * 128], ps[0:wdt, :])
            for m in range(4):
                for n2 in range(2):
                    ps = mp.tile([128, 512], f32, tag="mp")
                    for kk in range(4):
                        nc.tensor.matmul(ps, tw1t_s[:, kk, m * 128:(m + 1) * 128],
                                         rt[kk][:, n2 * 512:(n2 + 1) * 512],
                                         start=(kk == 0), stop=(kk == 3))
                    nc.scalar.activation(h1t[:, m, n2 * 512:(n2 + 1) * 512], ps,
                                         AF.Relu, bias=tb1_s[:, m:m + 1])
            for m in range(2):
                for n2 in range(2):
                    ps = mp.tile([128, 512], f32, tag="mp")
                    for kk in range(4):
                        nc.tensor.matmul(ps, tw2t_s[:, kk, m * 128:(m + 1) * 128],
                                         h1t[:, kk, n2 * 512:(n2 + 1) * 512],
                                         start=(kk == 0), stop=(kk == 3))
                    nc.scalar.activation(h2t[:, m, n2 * 512:(n2 + 1) * 512], ps,
                                         AF.Relu, bias=tb2_s[:, m:m + 1])
            for n2 in range(2):
                ps = mp.tile([128, 512], f32, tag="mp")
                for kk in range(2):
                    nc.tensor.matmul(ps[0:1, :], tw3t_s[:, kk, :],
                                     h2t[:, kk, n2 * 512:(n2 + 1) * 512],
                                     start=(kk == 0), stop=(kk == 1))
                nc.scalar.activation(p_s[:, n2 * 512:(n2 + 1) * 512], ps[0:1, :],
                                     AF.Sigmoid, bias=tb3_s)
            nc.sync.dma_start(out_d.rearrange("a b -> b a"), p_s)

    nc.compile()
    return nc


# ---------------------------------------------------------------- ntff shim
def _install_ntff_shim():
    """Provide antenv.axon_hooks so run_bass_kernel_spmd(trace=True) can pull
    NTFF profiles through libaxon_pjrt (module is absent in this image)."""
    import types
    if "antenv.axon_hooks" in sys.modules:
        return
    try:
        import antenv
        from trn_agent_boot.trn_boot import _ntff_profile_via_ctypes
    except Exception:
        return
    mod = types.ModuleType("antenv.axon_hooks")
    _state = {"hook": None}
    mod.set_axon_ntff_profile_hook = lambda h: _state.__setitem__("hook", h)
    mod.get_axon_ntff_profile_hook = lambda: _state["hook"]
    sys.modules["antenv.axon_hooks"] = mod
    antenv.axon_hooks = mod
    try:
        hook = _ntff_profile_via_ctypes("/opt/axon/libaxon_pjrt.so")
        mod.set_axon_ntff_profile_hook(hook)
    except Exception:
        pass


# ---------------------------------------------------------------- entry
def kernel(x, lS_i, lS_o, emb_tables,
           bot_W1, bot_b1, bot_W2, bot_b2, bot_W3, bot_b3,
           top_W1, top_b1, top_W2, top_b2, top_W3, top_b3):
    t0 = time.time()
    static, idx16_rep, bagrel = _prep(lS_i)
    t1 = time.time()

    nc = _build(static)
    t2 = time.time()

    tabf = np.ascontiguousarray(np.asarray(emb_tables, dtype=np.float32))
    xT = np.ascontiguousarray(np.asarray(x, np.float32).T)          # [13, B]
    iota = np.tile(np.arange(128, dtype=np.float16), (128, 1))
    w1t = np.ascontiguousarray(np.asarray(bot_W1, np.float32).T)    # [13, 512]
    w2t = np.ascontiguousarray(
        np.asarray(bot_W2, np.float32).T.reshape(4, 128, 256).transpose(1, 0, 2))
    w3t = np.ascontiguousarray(
        np.asarray(bot_W3, np.float32).T.reshape(2, 128, 64).transpose(1, 0, 2))
    b1 = np.ascontiguousarray(np.asarray(bot_b1, np.float32).reshape(4, 128).T)
    b2 = np.ascontiguousarray(np.asarray(bot_b2, np.float32).reshape(2, 128).T)
    b3 = np.asarray(bot_b3, np.float32).reshape(64, 1)
    tw1 = np.zeros((512, 512), np.float32)
    tw1[:415, :] = np.asarray(top_W1, np.float32).T
    tw1t = np.ascontiguousarray(tw1.reshape(4, 128, 512).transpose(1, 0, 2))
    tw2t = np.ascontiguousarray(
        np.asarray(top_W2, np.float32).T.reshape(4, 128, 256).transpose(1, 0, 2))
    tw3t = np.ascontiguousarray(
        np.asarray(top_W3, np.float32).T.reshape(2, 128, 1).transpose(1, 0, 2))
    tb1 = np.ascontiguousarray(np.asarray(top_b1, np.float32).reshape(4, 128).T)
    tb2 = np.ascontiguousarray(np.asarray(top_b2, np.float32).reshape(2, 128).T)
    tb3 = np.asarray(top_b3, np.float32).reshape(1, 1)

    in_maps = []
    for c in range(NCORE):
        in_maps.append({
            "tab": tabf,
            "idx16": np.ascontiguousarray(idx16_rep[c]),
            "bagrel": np.ascontiguousarray(bagrel[c]),
            "iotaf": iota,
            "xT": np.ascontiguousarray(xT[:, c * BC:(c + 1) * BC]),
            "w1t": w1t, "w2t": w2t, "w3t": w3t,
            "b1": b1, "b2": b2, "b3": b3,
            "tw1t": tw1t, "tw2t": tw2t, "tw3t": tw3t,
            "tb1": tb1, "tb2": tb2, "tb3": tb3,
        })

    from concourse.bass_utils import run_bass_kernel_spmd
    do_trace = bool(os.environ.get("DLRM_TRACE"))
    if do_trace:
        _install_ntff_shim()
    res = run_bass_kernel_spmd(nc, in_maps, core_ids=list(range(NCORE)),
                               trace=do_trace)
    t3 = time.time()
    if do_trace:
        with open("/tmp/dlrm_exec_ns.txt", "w") as f:
            f.write(str(res.exec_time_ns))
    print(f"[kernel] prep {t1-t0:.1f}s build+compile {t2-t1:.1f}s "
          f"run {t3-t2:.1f}s exec_ns={res.exec_time_ns}", file=sys.stderr)
    out = np.concatenate([r["out"] for r in res.results], axis=0)
    return out.astype(np.float32)



# revision 4
# speedup vs baseline: 2.4250x; 2.4250x over previous
"""DLRM forward (bottom MLP + 26-table EmbeddingBag + dot interaction + top MLP)
on 8 Trainium2 NeuronCores via Bass/Tile.

Sharding: batch-parallel. Each core handles 1024 of the 8192 samples and owns a
replicated copy of all 26 embedding tables in its HBM. No collectives.

Gather strategy: `dma_gather` (SWDGE) needs int16 row indices, so each 200000-row
table is addressed as 7 chunks of <=32768 rows. Per (table, chunk) the host
routes that chunk's indices into a compacted int16 stream (bag-sorted order is
preserved). Pooling of the gathered rows into per-bag sums runs on the
TensorEngine: for each 128-row gathered tile, a one-hot assignment matrix A
(built on-chip from a host-supplied relative-bag vector via is_equal against an
iota row) maps rows -> bags of one 128-bag window, accumulating in PSUM.

The SPMD program must be identical across cores, so tile counts per
(table, chunk) are padded to the max over the 8 cores and each tile emits
pooling matmuls for the union of bag-windows any core touches there; a core's
`bagrel` data zeroes the windows it does not use.
"""

import os
import sys
import time

import numpy as np

# ---------------------------------------------------------------- constants
B = 8192
L = 10
NT = 26
V = 200000
D = 64
NCORE = 8
BC = B // NCORE            # 1024 samples per core
SLOT = BC * L              # 10240 gathers per (core, table)
SCH = 32768                # chunk rows (int16-addressable)
NCH = (V + SCH - 1) // SCH  # 7
TILE = 128
TP = 16                    # max tiles per gather piece (ring slot)
JB = 16                    # A-matrix build batch (jobs)
NWIN = BC // TILE          # 8 bag windows per core
PAD_BAGREL = -512.0


# ---------------------------------------------------------------- host prep
def _prep(lS_i):
    """Compute the shared static structure + per-core device arrays."""
    lS = np.asarray(lS_i).astype(np.int64)

    seg_loc = {}
    seg_bag = {}
    nrows = np.zeros((NCORE, NT, NCH), np.int64)
    bag_of_pos = np.arange(SLOT, dtype=np.int64) // L
    for c in range(NCORE):
        for t in range(NT):
            idx = lS[t, c * SLOT:(c + 1) * SLOT].astype(np.int64)
            ch = idx >> 15
            order = np.argsort(ch, kind="stable")
            sidx = idx[order]
            sbag = bag_of_pos[order]
            sch = ch[order]
            bounds = np.searchsorted(sch, np.arange(NCH + 1))
            for k in range(NCH):
                lo, hi = bounds[k], bounds[k + 1]
                seg_loc[(c, t, k)] = (sidx[lo:hi] - (k << 15)).astype(np.int16)
                seg_bag[(c, t, k)] = sbag[lo:hi]
                nrows[c, t, k] = hi - lo

    T_tk = np.maximum(1, ((nrows.max(axis=0) + TILE - 1) // TILE)).astype(np.int64)

    # static walk: gather pieces + pooling jobs
    pieces = []        # (t, k, ntiles, idxcol0)
    piece_tile0 = []   # first tile index of the piece within its (t, k)
    jobs = []          # [t, w, piece_idx, tile_in_piece, start, stop]
    idxcols = 0
    first_last = {}
    for t in range(NT):
        for k in range(NCH):
            ntk = int(T_tk[t, k])
            tile0 = 0
            while tile0 < ntk:
                nt_p = min(TP, ntk - tile0)
                p_idx = len(pieces)
                pieces.append((t, k, nt_p, idxcols))
                piece_tile0.append(tile0)
                for i in range(nt_p):
                    gt = tile0 + i
                    wins = set()
                    for c in range(NCORE):
                        bags = seg_bag[(c, t, k)][gt * TILE:(gt + 1) * TILE]
                        if len(bags):
                            wins.update(np.unique(bags // TILE).tolist())
                    if not wins:
                        wins = {0}
                    for w in sorted(wins):
                        j = len(jobs)
                        jobs.append([t, int(w), p_idx, i, False, False])
                        if (t, w) not in first_last:
                            first_last[(t, w)] = [j, j]
                        else:
                            first_last[(t, w)][1] = j
                idxcols += nt_p * (TILE // 16)
                tile0 += nt_p
    for t in range(NT):
        for w in range(NWIN):
            assert (t, w) in first_last, (t, w)
    for (t, w), (f, l_) in first_last.items():
        jobs[f][4] = True
        jobs[l_][5] = True
    njobs = len(jobs)

    idx16 = np.zeros((NCORE, 16, idxcols), np.int16)
    bagrel = np.full((NCORE, TILE, njobs), PAD_BAGREL, np.float16)
    for c in range(NCORE):
        for p_idx, (t, k, nt_p, col0) in enumerate(pieces):
            tile0 = piece_tile0[p_idx]
            loc = seg_loc[(c, t, k)]
            n = len(loc)
            lo = tile0 * TILE
            hi = min(n, (tile0 + nt_p) * TILE)
            chunk = np.zeros(nt_p * TILE, np.int16)
            if hi > lo:
                chunk[: hi - lo] = loc[lo:hi]
            # element i -> [i % 16, i // 16]
            idx16[c, :, col0:col0 + nt_p * (TILE // 16)] = chunk.reshape(-1, 16).T

        for j, (t, w, p_idx, i, _s, _e) in enumerate(jobs):
            _t, k, nt_p, col0 = pieces[p_idx]
            gt = piece_tile0[p_idx] + i
            seg = seg_bag[(c, t, k)][gt * TILE:(gt + 1) * TILE]
            if len(seg):
                bagrel[c, : len(seg), j] = (
                    seg.astype(np.float32) - 128.0 * w).astype(np.float16)

    idx16_rep = np.tile(idx16, (1, 8, 1))      # replicate into 8 bands of 16
    static = dict(pieces=pieces, piece_tile0=piece_tile0, jobs=jobs,
                  idxcols=idxcols, njobs=njobs, T_tk=T_tk)
    return static, idx16_rep, bagrel


# ---------------------------------------------------------------- program
def _build(static):
    import concourse.tile as tile
    from concourse import bacc, mybir
    from concourse.masks import make_identity

    f32 = mybir.dt.float32
    bf16 = mybir.dt.bfloat16
    fp16 = mybir.dt.float16
    i16 = mybir.dt.int16
    AF = mybir.ActivationFunctionType
    ALU = mybir.AluOpType
    AX = mybir.AxisListType

    pieces = static["pieces"]
    jobs = static["jobs"]
    idxcols = static["idxcols"]
    njobs = static["njobs"]

    nc = bacc.Bacc("TRN2", target_bir_lowering=False, debug=False,
                   num_devices=NCORE, num_swdge_queues=4)

    tab = nc.dram_tensor("tab", [NT, V, D], f32, kind="ExternalInput").ap()
    idx16_d = nc.dram_tensor("idx16", [128, idxcols], i16, kind="ExternalInput").ap()
    bagrel_d = nc.dram_tensor("bagrel", [128, njobs], fp16, kind="ExternalInput").ap()
    iota_d = nc.dram_tensor("iotaf", [128, 128], fp16, kind="ExternalInput").ap()
    xT_d = nc.dram_tensor("xT", [13, BC], f32, kind="ExternalInput").ap()
    w1t_d = nc.dram_tensor("w1t", [13, 512], f32, kind="ExternalInput").ap()
    w2t_d = nc.dram_tensor("w2t", [128, 4, 256], f32, kind="ExternalInput").ap()
    w3t_d = nc.dram_tensor("w3t", [128, 2, 64], f32, kind="ExternalInput").ap()
    b1_d = nc.dram_tensor("b1", [128, 4], f32, kind="ExternalInput").ap()
    b2_d = nc.dram_tensor("b2", [128, 2], f32, kind="ExternalInput").ap()
    b3_d = nc.dram_tensor("b3", [64, 1], f32, kind="ExternalInput").ap()
    tw1t_d = nc.dram_tensor("tw1t", [128, 4, 512], f32, kind="ExternalInput").ap()
    tw2t_d = nc.dram_tensor("tw2t", [128, 4, 256], f32, kind="ExternalInput").ap()
    tw3t_d = nc.dram_tensor("tw3t", [128, 2, 1], f32, kind="ExternalInput").ap()
    tb1_d = nc.dram_tensor("tb1", [128, 4], f32, kind="ExternalInput").ap()
    tb2_d = nc.dram_tensor("tb2", [128, 2], f32, kind="ExternalInput").ap()
    tb3_d = nc.dram_tensor("tb3", [1, 1], f32, kind="ExternalInput").ap()
    out_d = nc.dram_tensor("out", [BC, 1], f32, kind="ExternalOutput").ap()

    idx16_s = nc.alloc_sbuf_tensor("idx16_s", [128, idxcols], i16).ap()
    bagrel_s = nc.alloc_sbuf_tensor("bagrel_s", [128, njobs], fp16).ap()
    iota_s = nc.alloc_sbuf_tensor("iota_s", [128, 128], fp16).ap()
    tall = nc.alloc_sbuf_tensor("tall", [128, NWIN, NT + 1, D], bf16).ap()
    r_all = nc.alloc_sbuf_tensor("r_all", [128, NWIN, 416], f32).ap()
    itmp = nc.alloc_sbuf_tensor("itmp", [128, NT, D], bf16).ap()
    xT_s = nc.alloc_sbuf_tensor("xT_s", [13, BC], f32).ap()
    w1t_s = nc.alloc_sbuf_tensor("w1t_s", [13, 512], f32).ap()
    w2t_s = nc.alloc_sbuf_tensor("w2t_s", [128, 4, 256], f32).ap()
    w3t_s = nc.alloc_sbuf_tensor("w3t_s", [128, 2, 64], f32).ap()
    b1_s = nc.alloc_sbuf_tensor("b1_s", [128, 4], f32).ap()
    b2_s = nc.alloc_sbuf_tensor("b2_s", [128, 2], f32).ap()
    b3_s = nc.alloc_sbuf_tensor("b3_s", [64, 1], f32).ap()
    tw1t_s = nc.alloc_sbuf_tensor("tw1t_s", [128, 4, 512], f32).ap()
    tw2t_s = nc.alloc_sbuf_tensor("tw2t_s", [128, 4, 256], f32).ap()
    tw3t_s = nc.alloc_sbuf_tensor("tw3t_s", [128, 2, 1], f32).ap()
    tb1_s = nc.alloc_sbuf_tensor("tb1_s", [128, 4], f32).ap()
    tb2_s = nc.alloc_sbuf_tensor("tb2_s", [128, 2], f32).ap()
    tb3_s = nc.alloc_sbuf_tensor("tb3_s", [1, 1], f32).ap()
    h1t = nc.alloc_sbuf_tensor("h1t", [128, 4, BC], f32).ap()
    h2t = nc.alloc_sbuf_tensor("h2t", [128, 2, BC], f32).ap()
    h3t = nc.alloc_sbuf_tensor("h3t", [64, BC], f32).ap()
    ident = nc.alloc_sbuf_tensor("ident", [128, 128], f32).ap()
    p_s = nc.alloc_sbuf_tensor("p_s", [1, BC], f32).ap()

    with tile.TileContext(nc) as tc:
        with (
            tc.tile_pool(name="g", bufs=10) as gp,
            tc.tile_pool(name="ab", bufs=3) as abp,
            tc.tile_pool(name="gbf", bufs=6) as gbfp,
            tc.tile_pool(name="pp", bufs=2, space="PSUM") as pp,
            tc.tile_pool(name="mp", bufs=2, space="PSUM") as mp,
            tc.tile_pool(name="tp", bufs=2, space="PSUM") as tpp,
        ):
            # ---- input loads
            nc.sync.dma_start(idx16_s, idx16_d)
            nc.sync.dma_start(bagrel_s, bagrel_d)
            nc.sync.dma_start(iota_s, iota_d)
            nc.sync.dma_start(xT_s, xT_d)
            for a, b_ in [(w1t_s, w1t_d), (w2t_s, w2t_d), (w3t_s, w3t_d),
                          (b1_s, b1_d), (b2_s, b2_d), (b3_s, b3_d),
                          (tw1t_s, tw1t_d), (tw2t_s, tw2t_d), (tw3t_s, tw3t_d),
                          (tb1_s, tb1_d), (tb2_s, tb2_d), (tb3_s, tb3_d)]:
                nc.sync.dma_start(a, b_)
            make_identity(nc, ident)
            nc.vector.memset(r_all, 0.0)
            nc.vector.memset(tall, 0.0)

            # ---- bottom MLP (no dependence on gathers; overlaps them)
            for m in range(4):
                for n2 in range(2):
                    ps = mp.tile([128, 512], f32, tag="mp")
                    nc.tensor.matmul(ps, w1t_s[:, m * 128:(m + 1) * 128],
                                     xT_s[:, n2 * 512:(n2 + 1) * 512],
                                     start=True, stop=True)
                    nc.scalar.activation(h1t[:, m, n2 * 512:(n2 + 1) * 512], ps,
                                         AF.Relu, bias=b1_s[:, m:m + 1])
            for m in range(2):
                for n2 in range(2):
                    ps = mp.tile([128, 512], f32, tag="mp")
                    for kk in range(4):
                        nc.tensor.matmul(ps, w2t_s[:, kk, m * 128:(m + 1) * 128],
                                         h1t[:, kk, n2 * 512:(n2 + 1) * 512],
                                         start=(kk == 0), stop=(kk == 3))
                    nc.scalar.activation(h2t[:, m, n2 * 512:(n2 + 1) * 512], ps,
                                         AF.Relu, bias=b2_s[:, m:m + 1])
            for n2 in range(2):
                ps = mp.tile([128, 512], f32, tag="mp")
                for kk in range(2):
                    nc.tensor.matmul(ps[0:64, :], w3t_s[:, kk, :],
                                     h2t[:, kk, n2 * 512:(n2 + 1) * 512],
                                     start=(kk == 0), stop=(kk == 1))
                nc.scalar.activation(h3t[:, n2 * 512:(n2 + 1) * 512], ps[0:64, :],
                                     AF.Relu, bias=b3_s)
            for blk in range(NWIN):
                ps = tpp.tile([128, 128], f32, tag="tp")
                nc.tensor.transpose(ps[:, 0:64],
                                    h3t[:, blk * 128:(blk + 1) * 128],
                                    ident[0:64, 0:64])
                nc.vector.tensor_copy(r_all[:, blk, 0:64], ps[:, 0:64])
                nc.scalar.activation(tall[:, blk, 0, :], ps[:, 0:64], AF.Copy)

            # ---- gather + pool
            np_lim = int(os.environ.get("DLRM_NPIECES", "1000000"))
            job_ptr = 0
            a_tile = None
            a_base = -1
            pooled = None
            cur_t = -1
            for p_idx, (t, k, nt_p, col0) in enumerate(pieces):
                if p_idx >= np_lim:
                    job_ptr = len(jobs)
                    break
                if t != cur_t:
                    if pooled is not None:
                        nc.scalar.activation(tall[:, :, cur_t + 1, :],
                                             pooled[:], AF.Copy)
                    pooled = pp.tile([128, NWIN, 64], f32, tag="pp")
                    cur_t = t
                rows = min(V - k * SCH, SCH)
                src = tab[t, k * SCH:k * SCH + rows, :]
                g = gp.tile([128, TP, 64], f32, tag="gbuf")
                n_idx = nt_p * TILE
                nc.gpsimd.dma_gather(
                    g[:, :nt_p, :], src,
                    idx16_s[:, col0:col0 + nt_p * 8],
                    num_idxs=n_idx, num_idxs_reg=n_idx,
                    elem_size=64, elem_step=64, single_packet=False,
                    queue_num=p_idx % 4,
                )
                gbf = gbfp.tile([128, TP, 64], bf16, tag="gbf")
                nc.scalar.activation(gbf[:, :nt_p, :], g[:, :nt_p, :], AF.Copy)
                while job_ptr < len(jobs) and jobs[job_ptr][2] == p_idx:
                    jt, w, _p, i, st, en = jobs[job_ptr]
                    if a_tile is None or job_ptr >= a_base + JB:
                        a_base = job_ptr
                        nb = min(JB, njobs - a_base)
                        a_tile = abp.tile([128, JB, 128], bf16, tag="ab")
                        nc.vector.tensor_tensor(
                            a_tile[:, :nb, :],
                            bagrel_s[:, a_base:a_base + nb, None]
                            .to_broadcast([128, nb, 128]),
                            iota_s[:, None, :].to_broadcast([128, nb, 128]),
                            op=ALU.is_equal,
                        )
                    nc.tensor.matmul(pooled[:, w, :],
                                     a_tile[:, job_ptr - a_base, :],
                                     gbf[:, i, :],
                                     start=st, stop=en, skip_group_check=True)
                    job_ptr += 1
            assert job_ptr == len(jobs)
            if pooled is not None:
                nc.scalar.activation(tall[:, :, cur_t + 1, :], pooled[:], AF.Copy)

            # ---- dot interaction (Z lower triangle -> r_all cols 64..415)
            off = 0
            for n in range(1, NT + 1):
                for blk in range(NWIN):
                    nc.vector.tensor_tensor(
                        itmp[:, 0:n, :],
                        tall[:, blk, n, None, :].to_broadcast([128, n, 64]),
                        tall[:, blk, 0:n, :],
                        op=ALU.mult,
                    )
                    nc.vector.tensor_reduce(
                        r_all[:, blk, 64 + off:64 + off + n],
                        itmp[:, 0:n, :], axis=AX.X, op=ALU.add,
                    )
                off += n

            # ---- top MLP (R^T tiles share the gather pool slots)
            rt = []
            for f in range(4):
                rt.append(gp.tile([128, 1024], f32, tag="gbuf", name=f"rt{f}"))
            nc.vector.memset(rt[3][:, :], 0.0)
            for f in range(4):
                wdt = 128 if f < 3 else 32
                for blk in range(NWIN):
                    ps = tpp.tile([128, 128], f32, tag="tp")
                    nc.tensor.transpose(ps[0:wdt, :],
                                        r_all[:, blk, f * 128:f * 128 + wdt],
                                        ident)
                    nc.vector.tensor_copy(
                        rt[f][0:wdt, blk * 128:(blk + 1) * 128], ps[0:wdt, :])
            for m in range(4):
                for n2 in range(2):
                    ps = mp.tile([128, 512], f32, tag="mp")
                    for kk in range(4):
                        nc.tensor.matmul(ps, tw1t_s[:, kk, m * 128:(m + 1) * 128],
                                         rt[kk][:, n2 * 512:(n2 + 1) * 512],
                                         start=(kk == 0), stop=(kk == 3))
                    nc.scalar.activation(h1t[:, m, n2 * 512:(n2 + 1) * 512], ps,
                                         AF.Relu, bias=tb1_s[:, m:m + 1])
            for m in range(2):
                for n2 in range(2):
                    ps = mp.tile([128, 512], f32, tag="mp")
                    for kk in range(4):
                        nc.tensor.matmul(ps, tw2t_s[:, kk, m * 128:(m + 1) * 128],
                                         h1t[:, kk, n2 * 512:(n2 + 1) * 512],
                                         start=(kk == 0), stop=(kk == 3))
                    nc.scalar.activation(h2t[:, m, n2 * 512:(n2 + 1) * 512], ps,
                                         AF.Relu, bias=tb2_s[:, m:m + 1])
            for n2 in range(2):
                ps = mp.tile([128, 512], f32, tag="mp")
                for kk in range(2):
                    nc.tensor.matmul(ps[0:1, :], tw3t_s[:, kk, :],
                                     h2t[:, kk, n2 * 512:(n2 + 1) * 512],
                                     start=(kk == 0), stop=(kk == 1))
                nc.scalar.activation(p_s[:, n2 * 512:(n2 + 1) * 512], ps[0:1, :],
                                     AF.Sigmoid, bias=tb3_s)
            nc.sync.dma_start(out_d.rearrange("a b -> b a"), p_s)

    nc.compile()
    return nc


# ---------------------------------------------------------------- ntff shim
def _install_ntff_shim():
    """Provide antenv.axon_hooks so run_bass_kernel_spmd(trace=True) can pull
    NTFF profiles through libaxon_pjrt (module is absent in this image)."""
    import types
    if "antenv.axon_hooks" in sys.modules:
        return
    try:
        import antenv
        from trn_agent_boot.trn_boot import _ntff_profile_via_ctypes
    except Exception:
        return
    mod = types.ModuleType("antenv.axon_hooks")
    _state = {"hook": None}
    mod.set_axon_ntff_profile_hook = lambda h: _state.__setitem__("hook", h)
    mod.get_axon_ntff_profile_hook = lambda: _state["hook"]
    sys.modules["antenv.axon_hooks"] = mod
    antenv.axon_hooks = mod
    try:
        hook = _ntff_profile_via_ctypes("/opt/axon/libaxon_pjrt.so")
        mod.set_axon_ntff_profile_hook(hook)
    except Exception:
        pass


# ---------------------------------------------------------------- entry
def kernel(x, lS_i, lS_o, emb_tables,
           bot_W1, bot_b1, bot_W2, bot_b2, bot_W3, bot_b3,
           top_W1, top_b1, top_W2, top_b2, top_W3, top_b3):
    t0 = time.time()
    static, idx16_rep, bagrel = _prep(lS_i)
    t1 = time.time()

    nc = _build(static)
    t2 = time.time()

    tabf = np.ascontiguousarray(np.asarray(emb_tables, dtype=np.float32))
    xT = np.ascontiguousarray(np.asarray(x, np.float32).T)          # [13, B]
    iota = np.tile(np.arange(128, dtype=np.float16), (128, 1))
    w1t = np.ascontiguousarray(np.asarray(bot_W1, np.float32).T)    # [13, 512]
    w2t = np.ascontiguousarray(
        np.asarray(bot_W2, np.float32).T.reshape(4, 128, 256).transpose(1, 0, 2))
    w3t = np.ascontiguousarray(
        np.asarray(bot_W3, np.float32).T.reshape(2, 128, 64).transpose(1, 0, 2))
    b1 = np.ascontiguousarray(np.asarray(bot_b1, np.float32).reshape(4, 128).T)
    b2 = np.ascontiguousarray(np.asarray(bot_b2, np.float32).reshape(2, 128).T)
    b3 = np.asarray(bot_b3, np.float32).reshape(64, 1)
    tw1 = np.zeros((512, 512), np.float32)
    tw1[:415, :] = np.asarray(top_W1, np.float32).T
    tw1t = np.ascontiguousarray(tw1.reshape(4, 128, 512).transpose(1, 0, 2))
    tw2t = np.ascontiguousarray(
        np.asarray(top_W2, np.float32).T.reshape(4, 128, 256).transpose(1, 0, 2))
    tw3t = np.ascontiguousarray(
        np.asarray(top_W3, np.float32).T.reshape(2, 128, 1).transpose(1, 0, 2))
    tb1 = np.ascontiguousarray(np.asarray(top_b1, np.float32).reshape(4, 128).T)
    tb2 = np.ascontiguousarray(np.asarray(top_b2, np.float32).reshape(2, 128).T)
    tb3 = np.asarray(top_b3, np.float32).reshape(1, 1)

    in_maps = []
    for c in range(NCORE):
        in_maps.append({
            "tab": tabf,
            "idx16": np.ascontiguousarray(idx16_rep[c]),
            "bagrel": np.ascontiguousarray(bagrel[c]),
            "iotaf": iota,
            "xT": np.ascontiguousarray(xT[:, c * BC:(c + 1) * BC]),
            "w1t": w1t, "w2t": w2t, "w3t": w3t,
            "b1": b1, "b2": b2, "b3": b3,
            "tw1t": tw1t, "tw2t": tw2t, "tw3t": tw3t,
            "tb1": tb1, "tb2": tb2, "tb3": tb3,
        })

    from concourse.bass_utils import run_bass_kernel_spmd
    do_trace = bool(os.environ.get("DLRM_TRACE"))
    if do_trace:
        _install_ntff_shim()
    res = run_bass_kernel_spmd(nc, in_maps, core_ids=list(range(NCORE)),
                               trace=do_trace)
    t3 = time.time()
    if do_trace:
        with open("/tmp/dlrm_exec_ns.txt", "w") as f:
            f.write(str(res.exec_time_ns))
    print(f"[kernel] prep {t1-t0:.1f}s build+compile {t2-t1:.1f}s "
          f"run {t3-t2:.1f}s exec_ns={res.exec_time_ns}", file=sys.stderr)
    out = np.concatenate([r["out"] for r in res.results], axis=0)
    return out.astype(np.float32)



# revision 15
# speedup vs baseline: 2.4319x; 1.0029x over previous
"""DLRM forward (bottom MLP + 26-table EmbeddingBag + dot interaction + top MLP)
on 8 Trainium2 NeuronCores via Bass/Tile.

Sharding: batch-parallel. Each core handles 1024 of the 8192 samples and owns a
replicated copy of all 26 embedding tables in its HBM. No collectives.

Gather strategy: `dma_gather` (SWDGE) needs int16 row indices, so each 200000-row
table is addressed as 7 chunks of <=32768 rows. Per (table, chunk) the host
routes that chunk's indices into a compacted int16 stream (bag-sorted order is
preserved). Gathers round-robin over the 4 SWDGE queues so descriptor
generation runs on all four Q7 core pairs concurrently.

Pooling runs transposed on the TensorEngine: per gathered 128-row tile, the
tile (cast to bf16) is the stationary operand and a narrow one-hot assignment
matrix A [128 rows, 32 bags] (built on-chip from a host-supplied relative-bag
vector) is the moving operand, accumulating pooledT [64 dims, 1024 bags] in
PSUM. Windows of 32 bags keep A small; window position is a free-dim offset.
pooledT is transposed back to sample-major via TensorE for the interaction.

The SPMD program must be identical across cores, so tile counts per
(table, chunk) are padded to the max over the 8 cores and each tile emits
pooling matmuls for the union of bag-windows any core touches there; a core's
`bagrel` data zeroes the windows it does not use.
"""

import os
import sys
import time

import numpy as np

# ---------------------------------------------------------------- constants
B = 8192
L = 10
NT = 26
V = 200000
D = 64
NCORE = 8
BC = B // NCORE            # 1024 samples per core
SLOT = BC * L              # 10240 gathers per (core, table)
SCH = 32768                # chunk rows (int16-addressable)
NCH = (V + SCH - 1) // SCH  # 7
TILE = 128
TP = 16                    # max tiles per gather piece (ring slot)
JB = 16                    # A-matrix build batch (jobs)
W = 64                     # bag window width (free-dim cols of A)
NW = BC // W               # 32 bag windows per core
NBLK = BC // 128           # 8 sample blocks for interaction
PAD_BAGREL = -512.0


# ---------------------------------------------------------------- host prep
def _prep(lS_i):
    """Compute the shared static structure + per-core device arrays."""
    lS = np.asarray(lS_i).astype(np.int64)

    seg_loc = {}
    seg_bag = {}
    nrows = np.zeros((NCORE, NT, NCH), np.int64)
    bag_of_pos = np.arange(SLOT, dtype=np.int64) // L
    for c in range(NCORE):
        for t in range(NT):
            idx = lS[t, c * SLOT:(c + 1) * SLOT].astype(np.int64)
            ch = idx >> 15
            order = np.argsort(ch, kind="stable")
            sidx = idx[order]
            sbag = bag_of_pos[order]
            sch = ch[order]
            bounds = np.searchsorted(sch, np.arange(NCH + 1))
            for k in range(NCH):
                lo, hi = bounds[k], bounds[k + 1]
                seg_loc[(c, t, k)] = (sidx[lo:hi] - (k << 15)).astype(np.int16)
                seg_bag[(c, t, k)] = sbag[lo:hi]
                nrows[c, t, k] = hi - lo

    T_tk = np.maximum(1, ((nrows.max(axis=0) + TILE - 1) // TILE)).astype(np.int64)

    # static walk: gather pieces + pooling jobs
    pieces = []        # (t, k, ntiles, idxcol0)
    piece_tile0 = []   # first tile index of the piece within its (t, k)
    jobs = []          # [t, w, piece_idx, tile_in_piece, start, stop]
    idxcols = 0
    first_last = {}
    for t in range(NT):
        for k in range(NCH):
            ntk = int(T_tk[t, k])
            tile0 = 0
            while tile0 < ntk:
                nt_p = min(TP, ntk - tile0)
                p_idx = len(pieces)
                pieces.append((t, k, nt_p, idxcols))
                piece_tile0.append(tile0)
                for i in range(nt_p):
                    gt = tile0 + i
                    wins = set()
                    for c in range(NCORE):
                        bags = seg_bag[(c, t, k)][gt * TILE:(gt + 1) * TILE]
                        if len(bags):
                            wins.update(np.unique(bags // W).tolist())
                    if not wins:
                        wins = {0}
                    for w in sorted(wins):
                        j = len(jobs)
                        jobs.append([t, int(w), p_idx, i, False, False])
                        if (t, w) not in first_last:
                            first_last[(t, w)] = [j, j]
                        else:
                            first_last[(t, w)][1] = j
                idxcols += nt_p * (TILE // 16)
                tile0 += nt_p
    for t in range(NT):
        for w in range(NW):
            assert (t, w) in first_last, (t, w)
    for (t, w), (f, l_) in first_last.items():
        jobs[f][4] = True
        jobs[l_][5] = True
    njobs = len(jobs)

    idx16 = np.zeros((NCORE, 16, idxcols), np.int16)
    bagrel = np.full((NCORE, TILE, njobs), PAD_BAGREL, np.float16)
    for c in range(NCORE):
        for p_idx, (t, k, nt_p, col0) in enumerate(pieces):
            tile0 = piece_tile0[p_idx]
            loc = seg_loc[(c, t, k)]
            n = len(loc)
            lo = tile0 * TILE
            hi = min(n, (tile0 + nt_p) * TILE)
            chunk = np.zeros(nt_p * TILE, np.int16)
            if hi > lo:
                chunk[: hi - lo] = loc[lo:hi]
            # element i -> [i % 16, i // 16]
            idx16[c, :, col0:col0 + nt_p * (TILE // 16)] = chunk.reshape(-1, 16).T

        for j, (t, w, p_idx, i, _s, _e) in enumerate(jobs):
            _t, k, nt_p, col0 = pieces[p_idx]
            gt = piece_tile0[p_idx] + i
            seg = seg_bag[(c, t, k)][gt * TILE:(gt + 1) * TILE]
            if len(seg):
                bagrel[c, : len(seg), j] = (
                    seg.astype(np.float32) - float(W) * w).astype(np.float16)

    idx16_rep = np.tile(idx16, (1, 8, 1))      # replicate into 8 bands of 16
    static = dict(pieces=pieces, piece_tile0=piece_tile0, jobs=jobs,
                  idxcols=idxcols, njobs=njobs, T_tk=T_tk)
    return static, idx16_rep, bagrel


# ---------------------------------------------------------------- program
def _build(static):
    import concourse.tile as tile
    from concourse import bacc, mybir
    from concourse.masks import make_identity

    f32 = mybir.dt.float32
    bf16 = mybir.dt.bfloat16
    fp16 = mybir.dt.float16
    i16 = mybir.dt.int16
    AF = mybir.ActivationFunctionType
    ALU = mybir.AluOpType
    AX = mybir.AxisListType

    pieces = static["pieces"]
    jobs = static["jobs"]
    idxcols = static["idxcols"]
    njobs = static["njobs"]

    nc = bacc.Bacc("TRN2", target_bir_lowering=False, debug=False,
                   num_devices=NCORE, num_swdge_queues=4)

    tab = nc.dram_tensor("tab", [NT, V, D], f32, kind="ExternalInput").ap()
    idx16_d = nc.dram_tensor("idx16", [128, idxcols], i16, kind="ExternalInput").ap()
    bagrel_d = nc.dram_tensor("bagrel", [128, njobs], fp16, kind="ExternalInput").ap()
    iota_d = nc.dram_tensor("iotaf", [128, 128], fp16, kind="ExternalInput").ap()
    xT_d = nc.dram_tensor("xT", [13, BC], f32, kind="ExternalInput").ap()
    w1t_d = nc.dram_tensor("w1t", [13, 512], f32, kind="ExternalInput").ap()
    w2t_d = nc.dram_tensor("w2t", [128, 4, 256], f32, kind="ExternalInput").ap()
    w3t_d = nc.dram_tensor("w3t", [128, 2, 64], f32, kind="ExternalInput").ap()
    b1_d = nc.dram_tensor("b1", [128, 4], f32, kind="ExternalInput").ap()
    b2_d = nc.dram_tensor("b2", [128, 2], f32, kind="ExternalInput").ap()
    b3_d = nc.dram_tensor("b3", [64, 1], f32, kind="ExternalInput").ap()
    tw1t_d = nc.dram_tensor("tw1t", [128, 4, 512], fp16, kind="ExternalInput").ap()
    tw2t_d = nc.dram_tensor("tw2t", [128, 4, 256], fp16, kind="ExternalInput").ap()
    tw3t_d = nc.dram_tensor("tw3t", [128, 2, 1], fp16, kind="ExternalInput").ap()
    tb1_d = nc.dram_tensor("tb1", [128, 4], f32, kind="ExternalInput").ap()
    tb2_d = nc.dram_tensor("tb2", [128, 2], f32, kind="ExternalInput").ap()
    tb3_d = nc.dram_tensor("tb3", [1, 1], f32, kind="ExternalInput").ap()
    out_d = nc.dram_tensor("out", [BC, 1], f32, kind="ExternalOutput").ap()

    idx16_s = nc.alloc_sbuf_tensor("idx16_s", [128, idxcols], i16).ap()
    bagrel_s = nc.alloc_sbuf_tensor("bagrel_s", [128, njobs], fp16).ap()
    iota_s = nc.alloc_sbuf_tensor("iota_s", [128, 128], fp16).ap()
    tall = nc.alloc_sbuf_tensor("tall", [128, NBLK, NT + 1, D], bf16).ap()
    r_all = nc.alloc_sbuf_tensor("r_all", [128, NBLK, 416], bf16).ap()
    itmp = nc.alloc_sbuf_tensor("itmp", [128, NT, D], bf16).ap()
    xT_s = nc.alloc_sbuf_tensor("xT_s", [13, BC], f32).ap()
    w1t_s = nc.alloc_sbuf_tensor("w1t_s", [13, 512], f32).ap()
    w2t_s = nc.alloc_sbuf_tensor("w2t_s", [128, 4, 256], f32).ap()
    w3t_s = nc.alloc_sbuf_tensor("w3t_s", [128, 2, 64], f32).ap()
    b1_s = nc.alloc_sbuf_tensor("b1_s", [128, 4], f32).ap()
    b2_s = nc.alloc_sbuf_tensor("b2_s", [128, 2], f32).ap()
    b3_s = nc.alloc_sbuf_tensor("b3_s", [64, 1], f32).ap()
    tw1t_s = nc.alloc_sbuf_tensor("tw1t_s", [128, 4, 512], fp16).ap()
    tw2t_s = nc.alloc_sbuf_tensor("tw2t_s", [128, 4, 256], fp16).ap()
    tw3t_s = nc.alloc_sbuf_tensor("tw3t_s", [128, 2, 1], fp16).ap()
    tb1_s = nc.alloc_sbuf_tensor("tb1_s", [128, 4], f32).ap()
    tb2_s = nc.alloc_sbuf_tensor("tb2_s", [128, 2], f32).ap()
    tb3_s = nc.alloc_sbuf_tensor("tb3_s", [1, 1], f32).ap()
    hb1 = nc.alloc_sbuf_tensor("hb1", [128, 4, BC], f32).ap()
    hb2 = nc.alloc_sbuf_tensor("hb2", [128, 2, BC], f32).ap()
    h3t = nc.alloc_sbuf_tensor("h3t", [64, BC], f32).ap()
    ht1 = nc.alloc_sbuf_tensor("ht1", [128, 4, BC], bf16).ap()
    ht2 = nc.alloc_sbuf_tensor("ht2", [128, 2, BC], bf16).ap()
    ident = nc.alloc_sbuf_tensor("ident", [128, 128], f32).ap()
    identb = nc.alloc_sbuf_tensor("identb", [128, 128], bf16).ap()
    p_s = nc.alloc_sbuf_tensor("p_s", [1, BC], f32).ap()

    with tile.TileContext(nc) as tc:
        with (
            tc.tile_pool(name="g", bufs=8) as gp,
            tc.tile_pool(name="ab", bufs=4) as abp,
            tc.tile_pool(name="gbf", bufs=6) as gbfp,
            tc.tile_pool(name="tb", bufs=2) as tbp,
            tc.tile_pool(name="pp", bufs=2, space="PSUM") as pp,
            tc.tile_pool(name="mp", bufs=1, space="PSUM") as mp,
            tc.tile_pool(name="tp", bufs=1, space="PSUM") as tpp,
            tc.tile_pool(name="tpb", bufs=2, space="PSUM") as tpbp,
        ):
            # ---- input loads
            nc.sync.dma_start(idx16_s, idx16_d)
            nc.sync.dma_start(bagrel_s, bagrel_d)
            nc.sync.dma_start(iota_s, iota_d)
            nc.sync.dma_start(xT_s, xT_d)
            for a, b_ in [(w1t_s, w1t_d), (w2t_s, w2t_d), (w3t_s, w3t_d),
                          (b1_s, b1_d), (b2_s, b2_d), (b3_s, b3_d),
                          (tw1t_s, tw1t_d), (tw2t_s, tw2t_d), (tw3t_s, tw3t_d),
                          (tb1_s, tb1_d), (tb2_s, tb2_d), (tb3_s, tb3_d)]:
                nc.sync.dma_start(a, b_)
            make_identity(nc, ident)
            nc.vector.tensor_copy(identb, ident)
            nc.vector.memset(r_all, 0.0)
            nc.vector.memset(tall, 0.0)

            # ---- bottom MLP (no dependence on gathers; overlaps them)
            for m in range(4):
                for n2 in range(2):
                    ps = mp.tile([128, 512], f32, tag="mp")
                    nc.tensor.matmul(ps, w1t_s[:, m * 128:(m + 1) * 128],
                                     xT_s[:, n2 * 512:(n2 + 1) * 512],
                                     start=True, stop=True)
                    nc.scalar.activation(hb1[:, m, n2 * 512:(n2 + 1) * 512], ps,
                                         AF.Relu, bias=b1_s[:, m:m + 1])
            for m in range(2):
                for n2 in range(2):
                    ps = mp.tile([128, 512], f32, tag="mp")
                    for kk in range(4):
                        nc.tensor.matmul(ps, w2t_s[:, kk, m * 128:(m + 1) * 128],
                                         hb1[:, kk, n2 * 512:(n2 + 1) * 512],
                                         start=(kk == 0), stop=(kk == 3))
                    nc.scalar.activation(hb2[:, m, n2 * 512:(n2 + 1) * 512], ps,
                                         AF.Relu, bias=b2_s[:, m:m + 1])
            for n2 in range(2):
                ps = mp.tile([128, 512], f32, tag="mp")
                for kk in range(2):
                    nc.tensor.matmul(ps[0:64, :], w3t_s[:, kk, :],
                                     hb2[:, kk, n2 * 512:(n2 + 1) * 512],
                                     start=(kk == 0), stop=(kk == 1))
                nc.scalar.activation(h3t[:, n2 * 512:(n2 + 1) * 512], ps[0:64, :],
                                     AF.Relu, bias=b3_s)
            for blk in range(NBLK):
                ps = tpp.tile([128, 128], f32, tag="tp")
                nc.tensor.transpose(ps[:, 0:64],
                                    h3t[:, blk * 128:(blk + 1) * 128],
                                    ident[0:64, 0:64])
                nc.vector.tensor_copy(r_all[:, blk, 0:64], ps[:, 0:64])
                nc.scalar.activation(tall[:, blk, 0, :], ps[:, 0:64], AF.Copy)

            # ---- gather + pool (transposed: gbf stationary, A moving)
            np_lim = int(os.environ.get("DLRM_NPIECES", "1000000"))
            job_ptr = 0
            a_tile = None
            a_base = -1
            pooledT = None
            cur_t = -1

            def finish_table(t):
                # PSUM pooledT [64, BC] -> SBUF bf16, then per-block transpose
                # back to sample-major tall[:, blk, t+1, :]
                tT = tbp.tile([64, BC], bf16, tag="tb")
                # two copies: a PSUM access pattern must stay within one
                # 2 KiB bank (BC/2 * 4 B = 2 KiB)
                nc.scalar.activation(tT[:, 0:BC // 2], pooledT[:, 0:BC // 2],
                                     AF.Copy)
                nc.scalar.activation(tT[:, BC // 2:], pooledT[:, BC // 2:],
                                     AF.Copy)
                for blk in range(NBLK):
                    tps = tpbp.tile([128, 128], bf16, tag="tpb")
                    nc.tensor.transpose(
                        tps[:, 0:64],
                        tT[:, blk * 128:(blk + 1) * 128],
                        identb[0:64, 0:64])
                    nc.scalar.activation(tall[:, blk, t + 1, :], tps[:, 0:64],
                                         AF.Copy)

            for p_idx, (t, k, nt_p, col0) in enumerate(pieces):
                if p_idx >= np_lim:
                    job_ptr = len(jobs)
                    break
                if t != cur_t:
                    if pooledT is not None:
                        finish_table(cur_t)
                    pooledT = pp.tile([64, BC], f32, tag="pp")
                    cur_t = t
                rows = min(V - k * SCH, SCH)
                src = tab[t, k * SCH:k * SCH + rows, :]
                g = gp.tile([128, TP, 64], f32, tag="gbuf")
                n_idx = nt_p * TILE
                nc.gpsimd.dma_gather(
                    g[:, :nt_p, :], src,
                    idx16_s[:, col0:col0 + nt_p * 8],
                    num_idxs=n_idx, num_idxs_reg=n_idx,
                    elem_size=64, elem_step=64, single_packet=False,
                    queue_num=p_idx % 4,
                )
                gbf = gbfp.tile([128, TP, 64], bf16, tag="gbf")
                nc.scalar.activation(gbf[:, :nt_p, :], g[:, :nt_p, :], AF.Copy)
                while job_ptr < len(jobs) and jobs[job_ptr][2] == p_idx:
                    jt, w, _p, i, st, en = jobs[job_ptr]
                    if a_tile is None or job_ptr >= a_base + JB:
                        a_base = job_ptr
                        nb = min(JB, njobs - a_base)
                        a_tile = abp.tile([128, JB, W], fp16, tag="ab")
                        nc.vector.tensor_tensor(
                            a_tile[:, :nb, :],
                            bagrel_s[:, a_base:a_base + nb, None]
                            .to_broadcast([128, nb, W]),
                            iota_s[:, None, 0:W].to_broadcast([128, nb, W]),
                            op=ALU.is_equal,
                        )
                    nc.tensor.matmul(pooledT[:, w * W:(w + 1) * W],
                                     gbf[:, i, :],
                                     a_tile[:, job_ptr - a_base, :],
                                     start=st, stop=en, skip_group_check=True)
                    job_ptr += 1
            assert job_ptr == len(jobs)
            if pooledT is not None:
                finish_table(cur_t)

            # ---- dot interaction (Z lower triangle -> r_all cols 64..415)
            with nc.allow_low_precision(reason="Z terms are 64-elem bf16 dots; "
                                        "feeds a bf16 top MLP within rel tol"):
                off = 0
                for n in range(1, NT + 1):
                    for blk in range(NBLK):
                        nc.vector.tensor_tensor(
                            itmp[:, 0:n, :],
                            tall[:, blk, n, None, :].to_broadcast([128, n, 64]),
                            tall[:, blk, 0:n, :],
                            op=ALU.mult,
                        )
                        nc.vector.tensor_reduce(
                            r_all[:, blk, 64 + off:64 + off + n],
                            itmp[:, 0:n, :], axis=AX.X, op=ALU.add,
                        )
                    off += n
                del off

            # ---- top MLP (R^T tiles share the gather pool slots)
            rt = []
            for f in range(4):
                rt.append(gp.tile([128, 1024], bf16, tag="gbuf", name=f"rt{f}"))
            nc.vector.memset(rt[3][:, :], 0.0)
            for f in range(4):
                wdt = 128 if f < 3 else 32
                for blk in range(NBLK):
                    ps = tpbp.tile([128, 128], bf16, tag="tpb")
                    nc.tensor.transpose(ps[0:wdt, :],
                                        r_all[:, blk, f * 128:f * 128 + wdt],
                                        identb)
                    nc.vector.tensor_copy(
                        rt[f][0:wdt, blk * 128:(blk + 1) * 128], ps[0:wdt, :])
            for m in range(4):
                for n2 in range(2):
                    ps = mp.tile([128, 512], f32, tag="mp")
                    for kk in range(4):
                        nc.tensor.matmul(ps, tw1t_s[:, kk, m * 128:(m + 1) * 128],
                                         rt[kk][:, n2 * 512:(n2 + 1) * 512],
                                         start=(kk == 0), stop=(kk == 3))
                    nc.scalar.activation(ht1[:, m, n2 * 512:(n2 + 1) * 512], ps,
                                         AF.Relu, bias=tb1_s[:, m:m + 1])
            for m in range(2):
                for n2 in range(2):
                    ps = mp.tile([128, 512], f32, tag="mp")
                    for kk in range(4):
                        nc.tensor.matmul(ps, tw2t_s[:, kk, m * 128:(m + 1) * 128],
                                         ht1[:, kk, n2 * 512:(n2 + 1) * 512],
                                         start=(kk == 0), stop=(kk == 3))
                    nc.scalar.activation(ht2[:, m, n2 * 512:(n2 + 1) * 512], ps,
                                         AF.Relu, bias=tb2_s[:, m:m + 1])
            for n2 in range(2):
                ps = mp.tile([128, 512], f32, tag="mp")
                for kk in range(2):
                    nc.tensor.matmul(ps[0:1, :], tw3t_s[:, kk, :],
                                     ht2[:, kk, n2 * 512:(n2 + 1) * 512],
                                     start=(kk == 0), stop=(kk == 1))
                nc.scalar.activation(p_s[:, n2 * 512:(n2 + 1) * 512], ps[0:1, :],
                                     AF.Sigmoid, bias=tb3_s)
            nc.sync.dma_start(out_d.rearrange("a b -> b a"), p_s)

    nc.compile()
    return nc


# ---------------------------------------------------------------- ntff shim
def _install_ntff_shim():
    """Provide antenv.axon_hooks so run_bass_kernel_spmd(trace=True) can pull
    NTFF profiles through libaxon_pjrt (module is absent in this image)."""
    import types
    if "antenv.axon_hooks" in sys.modules:
        return
    try:
        import antenv
        from trn_agent_boot.trn_boot import _ntff_profile_via_ctypes
    except Exception:
        return
    mod = types.ModuleType("antenv.axon_hooks")
    _state = {"hook": None}
    mod.set_axon_ntff_profile_hook = lambda h: _state.__setitem__("hook", h)
    mod.get_axon_ntff_profile_hook = lambda: _state["hook"]
    sys.modules["antenv.axon_hooks"] = mod
    antenv.axon_hooks = mod
    try:
        hook = _ntff_profile_via_ctypes("/opt/axon/libaxon_pjrt.so")
        mod.set_axon_ntff_profile_hook(hook)
    except Exception:
        pass


# ---------------------------------------------------------------- entry
def kernel(x, lS_i, lS_o, emb_tables,
           bot_W1, bot_b1, bot_W2, bot_b2, bot_W3, bot_b3,
           top_W1, top_b1, top_W2, top_b2, top_W3, top_b3):
    t0 = time.time()
    static, idx16_rep, bagrel = _prep(lS_i)
    t1 = time.time()

    nc = _build(static)
    t2 = time.time()

    tabf = np.ascontiguousarray(np.asarray(emb_tables, dtype=np.float32))
    xT = np.ascontiguousarray(np.asarray(x, np.float32).T)          # [13, B]
    iota = np.tile(np.arange(128, dtype=np.float16), (128, 1))
    w1t = np.ascontiguousarray(np.asarray(bot_W1, np.float32).T)    # [13, 512]
    w2t = np.ascontiguousarray(
        np.asarray(bot_W2, np.float32).T.reshape(4, 128, 256).transpose(1, 0, 2))
    w3t = np.ascontiguousarray(
        np.asarray(bot_W3, np.float32).T.reshape(2, 128, 64).transpose(1, 0, 2))
    b1 = np.ascontiguousarray(np.asarray(bot_b1, np.float32).reshape(4, 128).T)
    b2 = np.ascontiguousarray(np.asarray(bot_b2, np.float32).reshape(2, 128).T)
    b3 = np.asarray(bot_b3, np.float32).reshape(64, 1)
    tw1 = np.zeros((512, 512), np.float32)
    tw1[:415, :] = np.asarray(top_W1, np.float32).T
    tw1t = np.ascontiguousarray(
        tw1.reshape(4, 128, 512).transpose(1, 0, 2)).astype(np.float16)
    tw2t = np.ascontiguousarray(
        np.asarray(top_W2, np.float32).T.reshape(4, 128, 256)
        .transpose(1, 0, 2)).astype(np.float16)
    tw3t = np.ascontiguousarray(
        np.asarray(top_W3, np.float32).T.reshape(2, 128, 1)
        .transpose(1, 0, 2)).astype(np.float16)
    tb1 = np.ascontiguousarray(np.asarray(top_b1, np.float32).reshape(4, 128).T)
    tb2 = np.ascontiguousarray(np.asarray(top_b2, np.float32).reshape(2, 128).T)
    tb3 = np.asarray(top_b3, np.float32).reshape(1, 1)

    in_maps = []
    for c in range(NCORE):
        in_maps.append({
            "tab": tabf,
            "idx16": np.ascontiguousarray(idx16_rep[c]),
            "bagrel": np.ascontiguousarray(bagrel[c]),
            "iotaf": iota,
            "xT": np.ascontiguousarray(xT[:, c * BC:(c + 1) * BC]),
            "w1t": w1t, "w2t": w2t, "w3t": w3t,
            "b1": b1, "b2": b2, "b3": b3,
            "tw1t": tw1t, "tw2t": tw2t, "tw3t": tw3t,
            "tb1": tb1, "tb2": tb2, "tb3": tb3,
        })

    from concourse.bass_utils import run_bass_kernel_spmd
    do_trace = bool(os.environ.get("DLRM_TRACE"))
    if do_trace:
        _install_ntff_shim()
    res = run_bass_kernel_spmd(nc, in_maps, core_ids=list(range(NCORE)),
                               trace=do_trace)
    t3 = time.time()
    if do_trace:
        with open("/tmp/dlrm_exec_ns.txt", "w") as f:
            f.write(str(res.exec_time_ns))
    print(f"[kernel] prep {t1-t0:.1f}s build+compile {t2-t1:.1f}s "
          f"run {t3-t2:.1f}s exec_ns={res.exec_time_ns}", file=sys.stderr)
    out = np.concatenate([r["out"] for r in res.results], axis=0)
    return out.astype(np.float32)


# revision 16
# speedup vs baseline: 3.0080x; 1.2369x over previous
"""DLRM forward (bottom MLP + 26-table EmbeddingBag + dot interaction + top MLP)
on 8 Trainium2 NeuronCores via Bass/Tile.

Sharding: batch-parallel. Each core handles 1024 of the 8192 samples and owns a
replicated copy of all 26 embedding tables in its HBM. No collectives.

Gather: `dma_gather` (SWDGE, int16 indices) over 7 chunks of <=32768 rows per
table, round-robin across the 4 SWDGE queues so descriptor generation runs on
all four Q7 core pairs concurrently.

Pooling runs transposed on the TensorEngine: per gathered 128-row tile, the
tile (cast to bf16) is the stationary operand and a one-hot assignment matrix
A [128 rows, 128 bag-cols] is the moving operand, accumulating
pooledT [64 dims, 1024 bags] in PSUM (pre-zeroed per table; start=False).
A-matrices are data (a pure function of lS_i), so the host builds them in fp8
and they stream in over the otherwise-idle DMA engines -- no on-chip build.
Each job's 128-col window is tile-relative, with its base snapped so the
512 B PSUM output region never crosses a 2 KiB bank; tiles whose cross-core
bag-span cannot be covered by one allowed window get extra jobs (greedy cover).
pooledT transposes back to sample-major via TensorE for the interaction.
"""

import os
import sys
import time

import numpy as np

# ---------------------------------------------------------------- constants
B = 8192
L = 10
NT = 26
V = 200000
D = 64
NCORE = 8
BC = B // NCORE            # 1024 samples per core
SLOT = BC * L              # 10240 gathers per (core, table)
SCH = 32768                # chunk rows (int16-addressable)
NCH = (V + SCH - 1) // SCH  # 7
TILE = 128
TP = 16                    # max tiles per gather piece (ring slot)
JB = 16                    # A-matrix stream batch (jobs per DMA)
W = 128                    # bag window width (free-dim cols of A)
NBLK = BC // 128           # 8 sample blocks for interaction


def _snap(b):
    """Largest allowed base <= b: the [4b, 4b+512) PSUM region must stay in
    one 2 KiB bank, i.e. b mod 512 <= 384."""
    m = b % 512
    return b - (m - 384) if m > 384 else b


# ---------------------------------------------------------------- host prep
def _prep(lS_i):
    """Compute the shared static structure + per-core A-matrices."""
    lS = np.asarray(lS_i).astype(np.int64)

    seg_loc = {}
    seg_bag = {}
    nrows = np.zeros((NCORE, NT, NCH), np.int64)
    bag_of_pos = np.arange(SLOT, dtype=np.int64) // L
    for c in range(NCORE):
        for t in range(NT):
            idx = lS[t, c * SLOT:(c + 1) * SLOT].astype(np.int64)
            ch = idx >> 15
            order = np.argsort(ch, kind="stable")
            sidx = idx[order]
            sbag = bag_of_pos[order]
            sch = ch[order]
            bounds = np.searchsorted(sch, np.arange(NCH + 1))
            for k in range(NCH):
                lo, hi = bounds[k], bounds[k + 1]
                seg_loc[(c, t, k)] = (sidx[lo:hi] - (k << 15)).astype(np.int16)
                seg_bag[(c, t, k)] = sbag[lo:hi]
                nrows[c, t, k] = hi - lo

    T_tk = np.maximum(1, ((nrows.max(axis=0) + TILE - 1) // TILE)).astype(np.int64)

    # static walk: gather pieces + pooling jobs (greedy window cover per tile)
    pieces = []        # (t, k, ntiles, idxcol0)
    piece_tile0 = []   # first tile index of the piece within its (t, k)
    jobs = []          # [t, base, piece_idx, tile_in_piece]
    idxcols = 0
    last_job_of_table = {}
    for t in range(NT):
        for k in range(NCH):
            ntk = int(T_tk[t, k])
            tile0 = 0
            while tile0 < ntk:
                nt_p = min(TP, ntk - tile0)
                p_idx = len(pieces)
                pieces.append((t, k, nt_p, idxcols))
                piece_tile0.append(tile0)
                for i in range(nt_p):
                    gt = tile0 + i
                    bags = []
                    for c in range(NCORE):
                        s = seg_bag[(c, t, k)][gt * TILE:(gt + 1) * TILE]
                        if len(s):
                            bags.append([int(s.min()), int(s.max())])
                    if not bags:
                        jobs.append([t, 0, p_idx, i])
                        last_job_of_table[t] = len(jobs) - 1
                        continue
                    lo = min(b[0] for b in bags)
                    hi = max(b[1] for b in bags)
                    # greedy: snapped windows covering [lo, hi]
                    cur = lo
                    while True:
                        b = _snap(min(cur, BC - W))
                        jobs.append([t, b, p_idx, i])
                        last_job_of_table[t] = len(jobs) - 1
                        if b + W > hi:
                            break
                        cur = b + W
                idxcols += nt_p * (TILE // 16)
                tile0 += nt_p
    njobs = len(jobs)

    # Z column starts in r_all: each table's pair-block starts on an even
    # column so the bf16 reduce output stays 4-byte aligned
    zoff = []
    o = 64
    for n in range(1, NT + 1):
        zoff.append(o)
        o += n + (n & 1)
    assert o <= 512, o

    idx16 = np.zeros((NCORE, 16, idxcols), np.int16)
    f8 = None  # set by caller via pack_a
    for c in range(NCORE):
        for p_idx, (t, k, nt_p, col0) in enumerate(pieces):
            tile0 = piece_tile0[p_idx]
            loc = seg_loc[(c, t, k)]
            n = len(loc)
            lo = tile0 * TILE
            hi = min(n, (tile0 + nt_p) * TILE)
            chunk = np.zeros(nt_p * TILE, np.int16)
            if hi > lo:
                chunk[: hi - lo] = loc[lo:hi]
            # element i -> [i % 16, i // 16]
            idx16[c, :, col0:col0 + nt_p * (TILE // 16)] = chunk.reshape(-1, 16).T

    idx16_rep = np.tile(idx16, (1, 8, 1))      # replicate into 8 bands of 16
    static = dict(pieces=pieces, piece_tile0=piece_tile0, jobs=jobs,
                  idxcols=idxcols, njobs=njobs, T_tk=T_tk,
                  last_job_of_table=last_job_of_table,
                  seg_bag=seg_bag, zoff=zoff)
    return static, idx16_rep


def _pack_a(static, f8_dtype):
    """Per-core one-hot A matrices [128 rows, njobs, 128 cols] in fp8.
    Row r of job (tile) maps to col (bag - base) when it falls in the job's
    window; rows claimed by an earlier job of the same tile are left zero."""
    pieces = static["pieces"]
    piece_tile0 = static["piece_tile0"]
    jobs = static["jobs"]
    njobs = static["njobs"]
    seg_bag = static["seg_bag"]

    a_all = np.zeros((NCORE, TILE, njobs, W), f8_dtype)
    one = np.ones((), f8_dtype)
    for c in range(NCORE):
        prev_tile = None
        claimed = None
        for j, (t, base, p_idx, i) in enumerate(jobs):
            _t, k, nt_p, _c0 = pieces[p_idx]
            gt = piece_tile0[p_idx] + i
            seg = seg_bag[(c, t, k)][gt * TILE:(gt + 1) * TILE]
            if not len(seg):
                continue
            if prev_tile != (p_idx, i):
                prev_tile = (p_idx, i)
                claimed = np.zeros(len(seg), bool)
            rel = seg - base
            m = (rel >= 0) & (rel < W) & ~claimed
            claimed |= m
            rows = np.nonzero(m)[0]
            a_all[c, rows, j, rel[m]] = one
    return a_all


# ---------------------------------------------------------------- program
def _build(static):
    import concourse.tile as tile
    from concourse import bacc, mybir
    from concourse.masks import make_identity

    f32 = mybir.dt.float32
    bf16 = mybir.dt.bfloat16
    fp16 = mybir.dt.float16
    f8 = mybir.dt.float8e4
    i16 = mybir.dt.int16
    AF = mybir.ActivationFunctionType
    ALU = mybir.AluOpType
    AX = mybir.AxisListType

    pieces = static["pieces"]
    jobs = static["jobs"]
    idxcols = static["idxcols"]
    njobs = static["njobs"]
    last_job_of_table = static["last_job_of_table"]
    zoff = static["zoff"]

    nc = bacc.Bacc("TRN2", target_bir_lowering=False, debug=False,
                   num_devices=NCORE, num_swdge_queues=4)

    tab = nc.dram_tensor("tab", [NT, V, D], f32, kind="ExternalInput").ap()
    idx16_d = nc.dram_tensor("idx16", [128, idxcols], i16, kind="ExternalInput").ap()
    aall_d = nc.dram_tensor("aall", [128, njobs, W], f8, kind="ExternalInput").ap()
    xT_d = nc.dram_tensor("xT", [13, BC], f32, kind="ExternalInput").ap()
    w1t_d = nc.dram_tensor("w1t", [13, 512], f32, kind="ExternalInput").ap()
    w2t_d = nc.dram_tensor("w2t", [128, 4, 256], f32, kind="ExternalInput").ap()
    w3t_d = nc.dram_tensor("w3t", [128, 2, 64], f32, kind="ExternalInput").ap()
    b1_d = nc.dram_tensor("b1", [128, 4], f32, kind="ExternalInput").ap()
    b2_d = nc.dram_tensor("b2", [128, 2], f32, kind="ExternalInput").ap()
    b3_d = nc.dram_tensor("b3", [64, 1], f32, kind="ExternalInput").ap()
    tw1t_d = nc.dram_tensor("tw1t", [128, 4, 512], fp16, kind="ExternalInput").ap()
    tw2t_d = nc.dram_tensor("tw2t", [128, 4, 256], fp16, kind="ExternalInput").ap()
    tw3t_d = nc.dram_tensor("tw3t", [128, 2, 1], fp16, kind="ExternalInput").ap()
    tb1_d = nc.dram_tensor("tb1", [128, 4], f32, kind="ExternalInput").ap()
    tb2_d = nc.dram_tensor("tb2", [128, 2], f32, kind="ExternalInput").ap()
    tb3_d = nc.dram_tensor("tb3", [1, 1], f32, kind="ExternalInput").ap()
    out_d = nc.dram_tensor("out", [BC, 1], f32, kind="ExternalOutput").ap()

    idx16_s = nc.alloc_sbuf_tensor("idx16_s", [128, idxcols], i16).ap()
    tall = nc.alloc_sbuf_tensor("tall", [128, NBLK, NT + 1, D], bf16).ap()
    r_all = nc.alloc_sbuf_tensor("r_all", [128, NBLK, 512], bf16).ap()
    itmp = nc.alloc_sbuf_tensor("itmp", [128, NT, D], bf16).ap()
    xT_s = nc.alloc_sbuf_tensor("xT_s", [13, BC], f32).ap()
    w1t_s = nc.alloc_sbuf_tensor("w1t_s", [13, 512], f32).ap()
    w2t_s = nc.alloc_sbuf_tensor("w2t_s", [128, 4, 256], f32).ap()
    w3t_s = nc.alloc_sbuf_tensor("w3t_s", [128, 2, 64], f32).ap()
    b1_s = nc.alloc_sbuf_tensor("b1_s", [128, 4], f32).ap()
    b2_s = nc.alloc_sbuf_tensor("b2_s", [128, 2], f32).ap()
    b3_s = nc.alloc_sbuf_tensor("b3_s", [64, 1], f32).ap()
    tw1t_s = nc.alloc_sbuf_tensor("tw1t_s", [128, 4, 512], fp16).ap()
    tw2t_s = nc.alloc_sbuf_tensor("tw2t_s", [128, 4, 256], fp16).ap()
    tw3t_s = nc.alloc_sbuf_tensor("tw3t_s", [128, 2, 1], fp16).ap()
    tb1_s = nc.alloc_sbuf_tensor("tb1_s", [128, 4], f32).ap()
    tb2_s = nc.alloc_sbuf_tensor("tb2_s", [128, 2], f32).ap()
    tb3_s = nc.alloc_sbuf_tensor("tb3_s", [1, 1], f32).ap()
    hb1 = nc.alloc_sbuf_tensor("hb1", [128, 4, BC], f32).ap()
    hb2 = nc.alloc_sbuf_tensor("hb2", [128, 2, BC], f32).ap()
    h3t = nc.alloc_sbuf_tensor("h3t", [64, BC], f32).ap()
    ht1 = nc.alloc_sbuf_tensor("ht1", [128, 4, BC], bf16).ap()
    ht2 = nc.alloc_sbuf_tensor("ht2", [128, 2, BC], bf16).ap()
    ident = nc.alloc_sbuf_tensor("ident", [128, 128], f32).ap()
    identb = nc.alloc_sbuf_tensor("identb", [128, 128], bf16).ap()
    p_s = nc.alloc_sbuf_tensor("p_s", [1, BC], f32).ap()

    with tile.TileContext(nc) as tc:
        with (
            tc.tile_pool(name="g", bufs=8) as gp,
            tc.tile_pool(name="ab", bufs=4) as abp,
            tc.tile_pool(name="gbf", bufs=6) as gbfp,
            tc.tile_pool(name="tb", bufs=2) as tbp,
            tc.tile_pool(name="pp", bufs=2, space="PSUM") as pp,
            tc.tile_pool(name="mp", bufs=1, space="PSUM") as mp,
            tc.tile_pool(name="tp", bufs=1, space="PSUM") as tpp,
            tc.tile_pool(name="tpb", bufs=2, space="PSUM") as tpbp,
        ):
            # ---- input loads
            nc.sync.dma_start(idx16_s, idx16_d)
            nc.sync.dma_start(xT_s, xT_d)
            for a, b_ in [(w1t_s, w1t_d), (w2t_s, w2t_d), (w3t_s, w3t_d),
                          (b1_s, b1_d), (b2_s, b2_d), (b3_s, b3_d),
                          (tw1t_s, tw1t_d), (tw2t_s, tw2t_d), (tw3t_s, tw3t_d),
                          (tb1_s, tb1_d), (tb2_s, tb2_d), (tb3_s, tb3_d)]:
                nc.sync.dma_start(a, b_)
            make_identity(nc, ident)
            nc.vector.tensor_copy(identb, ident)
            nc.vector.memset(r_all, 0.0)
            nc.vector.memset(tall, 0.0)

            # ---- bottom MLP (no dependence on gathers; overlaps them)
            for m in range(4):
                for n2 in range(2):
                    ps = mp.tile([128, 512], f32, tag="mp")
                    nc.tensor.matmul(ps, w1t_s[:, m * 128:(m + 1) * 128],
                                     xT_s[:, n2 * 512:(n2 + 1) * 512],
                                     start=True, stop=True)
                    nc.scalar.activation(hb1[:, m, n2 * 512:(n2 + 1) * 512], ps,
                                         AF.Relu, bias=b1_s[:, m:m + 1])
            for m in range(2):
                for n2 in range(2):
                    ps = mp.tile([128, 512], f32, tag="mp")
                    for kk in range(4):
                        nc.tensor.matmul(ps, w2t_s[:, kk, m * 128:(m + 1) * 128],
                                         hb1[:, kk, n2 * 512:(n2 + 1) * 512],
                                         start=(kk == 0), stop=(kk == 3))
                    nc.scalar.activation(hb2[:, m, n2 * 512:(n2 + 1) * 512], ps,
                                         AF.Relu, bias=b2_s[:, m:m + 1])
            for n2 in range(2):
                ps = mp.tile([128, 512], f32, tag="mp")
                for kk in range(2):
                    nc.tensor.matmul(ps[0:64, :], w3t_s[:, kk, :],
                                     hb2[:, kk, n2 * 512:(n2 + 1) * 512],
                                     start=(kk == 0), stop=(kk == 1))
                nc.scalar.activation(h3t[:, n2 * 512:(n2 + 1) * 512], ps[0:64, :],
                                     AF.Relu, bias=b3_s)
            for blk in range(NBLK):
                ps = tpp.tile([128, 128], f32, tag="tp")
                nc.tensor.transpose(ps[:, 0:64],
                                    h3t[:, blk * 128:(blk + 1) * 128],
                                    ident[0:64, 0:64])
                nc.vector.tensor_copy(r_all[:, blk, 0:64], ps[:, 0:64])
                nc.scalar.activation(tall[:, blk, 0, :], ps[:, 0:64], AF.Copy)

            # ---- gather + pool (transposed: gbf stationary, fp8 A moving)
            job_ptr = 0
            a_tile = None
            a_base = -1
            pooledT = None
            cur_t = -1

            def finish_table(t):
                # PSUM pooledT [64, BC] -> SBUF bf16 (bank-sized halves),
                # then per-block transpose to sample-major tall[:, blk, t+1, :]
                tT = tbp.tile([64, BC], bf16, tag="tb")
                nc.scalar.activation(tT[:, 0:BC // 2], pooledT[:, 0:BC // 2],
                                     AF.Copy)
                nc.scalar.activation(tT[:, BC // 2:], pooledT[:, BC // 2:],
                                     AF.Copy)
                for blk in range(NBLK):
                    tps = tpbp.tile([128, 128], bf16, tag="tpb")
                    nc.tensor.transpose(
                        tps[:, 0:64],
                        tT[:, blk * 128:(blk + 1) * 128],
                        identb[0:64, 0:64])
                    nc.scalar.activation(tall[:, blk, t + 1, :], tps[:, 0:64],
                                         AF.Copy)
                # emit table t+1's interaction terms now so the Vector queue
                # interleaves them with the remaining tables' gathers
                interaction_chunk(t + 1)

            def interaction_chunk(n):
                with nc.allow_low_precision(reason="64-elem bf16 dots into "
                                            "a bf16 top MLP; within rel tol"):
                    for blk in range(NBLK):
                        nc.vector.tensor_tensor(
                            itmp[:, 0:n, :],
                            tall[:, blk, n, None, :].to_broadcast([128, n, 64]),
                            tall[:, blk, 0:n, :],
                            op=ALU.mult,
                        )
                        nc.vector.tensor_reduce(
                            r_all[:, blk, zoff[n - 1]:zoff[n - 1] + n],
                            itmp[:, 0:n, :], axis=AX.X, op=ALU.add,
                        )

            for p_idx, (t, k, nt_p, col0) in enumerate(pieces):
                if t != cur_t:
                    if pooledT is not None:
                        finish_table(cur_t)
                    pooledT = pp.tile([64, BC], f32, tag="pp")
                    # zero (bank-sized halves); matmuls accumulate start=False
                    nc.vector.memset(pooledT[:, 0:BC // 2], 0.0)
                    nc.vector.memset(pooledT[:, BC // 2:], 0.0)
                    cur_t = t
                rows = min(V - k * SCH, SCH)
                src = tab[t, k * SCH:k * SCH + rows, :]
                g = gp.tile([128, TP, 64], f32, tag="gbuf")
                n_idx = nt_p * TILE
                nc.gpsimd.dma_gather(
                    g[:, :nt_p, :], src,
                    idx16_s[:, col0:col0 + nt_p * 8],
                    num_idxs=n_idx, num_idxs_reg=n_idx,
                    elem_size=64, elem_step=64, single_packet=False,
                    queue_num=p_idx % 4,
                )
                gbf = gbfp.tile([128, TP, 64], bf16, tag="gbf")
                nc.scalar.activation(gbf[:, :nt_p, :], g[:, :nt_p, :], AF.Copy)
                while job_ptr < len(jobs) and jobs[job_ptr][2] == p_idx:
                    jt, base, _p, i = jobs[job_ptr]
                    if a_tile is None or job_ptr >= a_base + JB:
                        a_base = job_ptr
                        nb = min(JB, njobs - a_base)
                        a_tile = abp.tile([128, JB, W], f8, tag="ab")
                        nc.sync.dma_start(a_tile[:, :nb, :],
                                          aall_d[:, a_base:a_base + nb, :])
                    nc.tensor.matmul(pooledT[:, base:base + W],
                                     gbf[:, i, :],
                                     a_tile[:, job_ptr - a_base, :],
                                     start=False,
                                     stop=(job_ptr == last_job_of_table[jt]),
                                     skip_group_check=True)
                    job_ptr += 1
            assert job_ptr == len(jobs)
            if pooledT is not None:
                finish_table(cur_t)

            # ---- top MLP (R^T tiles share the gather pool slots)
            rt = []
            for f in range(4):
                rt.append(gp.tile([128, 1024], bf16, tag="gbuf", name=f"rt{f}"))
            nc.vector.memset(rt[3][:, :], 0.0)
            for f in range(4):
                wdt = 128
                for blk in range(NBLK):
                    ps = tpbp.tile([128, 128], bf16, tag="tpb")
                    nc.tensor.transpose(ps[0:wdt, :],
                                        r_all[:, blk, f * 128:f * 128 + wdt],
                                        identb)
                    nc.vector.tensor_copy(
                        rt[f][0:wdt, blk * 128:(blk + 1) * 128], ps[0:wdt, :])
            for m in range(4):
                for n2 in range(2):
                    ps = mp.tile([128, 512], f32, tag="mp")
                    for kk in range(4):
                        nc.tensor.matmul(ps, tw1t_s[:, kk, m * 128:(m + 1) * 128],
                                         rt[kk][:, n2 * 512:(n2 + 1) * 512],
                                         start=(kk == 0), stop=(kk == 3))
                    nc.scalar.activation(ht1[:, m, n2 * 512:(n2 + 1) * 512], ps,
                                         AF.Relu, bias=tb1_s[:, m:m + 1])
            for m in range(2):
                for n2 in range(2):
                    ps = mp.tile([128, 512], f32, tag="mp")
                    for kk in range(4):
                        nc.tensor.matmul(ps, tw2t_s[:, kk, m * 128:(m + 1) * 128],
                                         ht1[:, kk, n2 * 512:(n2 + 1) * 512],
                                         start=(kk == 0), stop=(kk == 3))
                    nc.scalar.activation(ht2[:, m, n2 * 512:(n2 + 1) * 512], ps,
                                         AF.Relu, bias=tb2_s[:, m:m + 1])
            for n2 in range(2):
                ps = mp.tile([128, 512], f32, tag="mp")
                for kk in range(2):
                    nc.tensor.matmul(ps[0:1, :], tw3t_s[:, kk, :],
                                     ht2[:, kk, n2 * 512:(n2 + 1) * 512],
                                     start=(kk == 0), stop=(kk == 1))
                nc.scalar.activation(p_s[:, n2 * 512:(n2 + 1) * 512], ps[0:1, :],
                                     AF.Sigmoid, bias=tb3_s)
            nc.sync.dma_start(out_d.rearrange("a b -> b a"), p_s)

    nc.compile()
    return nc


# ---------------------------------------------------------------- ntff shim
def _install_ntff_shim():
    """Provide antenv.axon_hooks so run_bass_kernel_spmd(trace=True) can pull
    NTFF profiles through libaxon_pjrt (module is absent in this image)."""
    import types
    if "antenv.axon_hooks" in sys.modules:
        return
    try:
        import antenv
        from trn_agent_boot.trn_boot import _ntff_profile_via_ctypes
    except Exception:
        return
    mod = types.ModuleType("antenv.axon_hooks")
    _state = {"hook": None}
    mod.set_axon_ntff_profile_hook = lambda h: _state.__setitem__("hook", h)
    mod.get_axon_ntff_profile_hook = lambda: _state["hook"]
    sys.modules["antenv.axon_hooks"] = mod
    antenv.axon_hooks = mod
    try:
        hook = _ntff_profile_via_ctypes("/opt/axon/libaxon_pjrt.so")
        mod.set_axon_ntff_profile_hook(hook)
    except Exception:
        pass


# ---------------------------------------------------------------- entry
def kernel(x, lS_i, lS_o, emb_tables,
           bot_W1, bot_b1, bot_W2, bot_b2, bot_W3, bot_b3,
           top_W1, top_b1, top_W2, top_b2, top_W3, top_b3):
    from concourse import mybir

    t0 = time.time()
    static, idx16_rep = _prep(lS_i)
    a_all = _pack_a(static, mybir.dt.np(mybir.dt.float8e4))
    t1 = time.time()

    nc = _build(static)
    t2 = time.time()

    tabf = np.ascontiguousarray(np.asarray(emb_tables, dtype=np.float32))
    xT = np.ascontiguousarray(np.asarray(x, np.float32).T)          # [13, B]
    w1t = np.ascontiguousarray(np.asarray(bot_W1, np.float32).T)    # [13, 512]
    w2t = np.ascontiguousarray(
        np.asarray(bot_W2, np.float32).T.reshape(4, 128, 256).transpose(1, 0, 2))
    w3t = np.ascontiguousarray(
        np.asarray(bot_W3, np.float32).T.reshape(2, 128, 64).transpose(1, 0, 2))
    b1 = np.ascontiguousarray(np.asarray(bot_b1, np.float32).reshape(4, 128).T)
    b2 = np.ascontiguousarray(np.asarray(bot_b2, np.float32).reshape(2, 128).T)
    b3 = np.asarray(bot_b3, np.float32).reshape(64, 1)
    tw1 = np.zeros((512, 512), np.float32)
    w1T = np.asarray(top_W1, np.float32).T    # [415, 512]
    tw1[0:64, :] = w1T[0:64, :]
    zoff = static["zoff"]
    off = 64
    for n in range(1, NT + 1):
        tw1[zoff[n - 1]:zoff[n - 1] + n, :] = w1T[off:off + n, :]
        off += n
    tw1t = np.ascontiguousarray(
        tw1.reshape(4, 128, 512).transpose(1, 0, 2)).astype(np.float16)
    tw2t = np.ascontiguousarray(
        np.asarray(top_W2, np.float32).T.reshape(4, 128, 256)
        .transpose(1, 0, 2)).astype(np.float16)
    tw3t = np.ascontiguousarray(
        np.asarray(top_W3, np.float32).T.reshape(2, 128, 1)
        .transpose(1, 0, 2)).astype(np.float16)
    tb1 = np.ascontiguousarray(np.asarray(top_b1, np.float32).reshape(4, 128).T)
    tb2 = np.ascontiguousarray(np.asarray(top_b2, np.float32).reshape(2, 128).T)
    tb3 = np.asarray(top_b3, np.float32).reshape(1, 1)

    in_maps = []
    for c in range(NCORE):
        in_maps.append({
            "tab": tabf,
            "idx16": np.ascontiguousarray(idx16_rep[c]),
            "aall": np.ascontiguousarray(a_all[c]),
            "xT": np.ascontiguousarray(xT[:, c * BC:(c + 1) * BC]),
            "w1t": w1t, "w2t": w2t, "w3t": w3t,
            "b1": b1, "b2": b2, "b3": b3,
            "tw1t": tw1t, "tw2t": tw2t, "tw3t": tw3t,
            "tb1": tb1, "tb2": tb2, "tb3": tb3,
        })

    from concourse.bass_utils import run_bass_kernel_spmd
    do_trace = bool(os.environ.get("DLRM_TRACE"))
    if do_trace:
        _install_ntff_shim()
    res = run_bass_kernel_spmd(nc, in_maps, core_ids=list(range(NCORE)),
                               trace=do_trace)
    t3 = time.time()
    if do_trace:
        with open("/tmp/dlrm_exec_ns.txt", "w") as f:
            f.write(str(res.exec_time_ns))
    print(f"[kernel] prep {t1-t0:.1f}s build+compile {t2-t1:.1f}s "
          f"run {t3-t2:.1f}s exec_ns={res.exec_time_ns}", file=sys.stderr)
    out = np.concatenate([r["out"] for r in res.results], axis=0)
    return out.astype(np.float32)


# revision 17
# speedup vs baseline: 3.0582x; 1.0167x over previous
"""DLRM forward (bottom MLP + 26-table EmbeddingBag + dot interaction + top MLP)
on 8 Trainium2 NeuronCores via Bass/Tile.

Sharding: batch-parallel. Each core handles 1024 of the 8192 samples and owns a
replicated copy of all 26 embedding tables in its HBM. No collectives.

Gather: `dma_gather` (SWDGE, int16 indices) over 7 chunks of <=32768 rows per
table, round-robin across the 4 SWDGE queues so descriptor generation runs on
all four Q7 core pairs concurrently.

Pooling runs transposed on the TensorEngine: per gathered 128-row tile, the
tile (cast to bf16) is the stationary operand and a one-hot assignment matrix
A [128 rows, 128 bag-cols] is the moving operand, accumulating
pooledT [64 dims, 1024 bags] in PSUM (pre-zeroed per table; start=False).
A-matrices are data (a pure function of lS_i), so the host builds them in fp8
and they stream in over the otherwise-idle DMA engines -- no on-chip build.
Each job's 128-col window is tile-relative, with its base snapped so the
512 B PSUM output region never crosses a 2 KiB bank; tiles whose cross-core
bag-span cannot be covered by one allowed window get extra jobs (greedy cover).
pooledT transposes back to sample-major via TensorE for the interaction.
"""

import os
import sys
import time

import numpy as np

# ---------------------------------------------------------------- constants
B = 8192
L = 10
NT = 26
V = 200000
D = 64
NCORE = 8
BC = B // NCORE            # 1024 samples per core
SLOT = BC * L              # 10240 gathers per (core, table)
SCH = 32768                # chunk rows (int16-addressable)
NCH = (V + SCH - 1) // SCH  # 7
TILE = 128
TP = 16                    # max tiles per gather piece (ring slot)
JB = 32                    # A-matrix stream batch (jobs per DMA)
W = 128                    # bag window width (free-dim cols of A)
NBLK = BC // 128           # 8 sample blocks for interaction


def _snap(b):
    """Largest allowed base <= b: the [4b, 4b+512) PSUM region must stay in
    one 2 KiB bank, i.e. b mod 512 <= 384."""
    m = b % 512
    return b - (m - 384) if m > 384 else b


# ---------------------------------------------------------------- host prep
def _prep(lS_i):
    """Compute the shared static structure + per-core A-matrices."""
    lS = np.asarray(lS_i).astype(np.int64)

    seg_loc = {}
    seg_bag = {}
    nrows = np.zeros((NCORE, NT, NCH), np.int64)
    bag_of_pos = np.arange(SLOT, dtype=np.int64) // L
    for c in range(NCORE):
        for t in range(NT):
            idx = lS[t, c * SLOT:(c + 1) * SLOT].astype(np.int64)
            ch = idx >> 15
            order = np.argsort(ch, kind="stable")
            sidx = idx[order]
            sbag = bag_of_pos[order]
            sch = ch[order]
            bounds = np.searchsorted(sch, np.arange(NCH + 1))
            for k in range(NCH):
                lo, hi = bounds[k], bounds[k + 1]
                seg_loc[(c, t, k)] = (sidx[lo:hi] - (k << 15)).astype(np.int16)
                seg_bag[(c, t, k)] = sbag[lo:hi]
                nrows[c, t, k] = hi - lo

    T_tk = np.maximum(1, ((nrows.max(axis=0) + TILE - 1) // TILE)).astype(np.int64)

    # static walk: gather pieces + pooling jobs (greedy window cover per tile)
    pieces = []        # (t, k, ntiles, idxcol0)
    piece_tile0 = []   # first tile index of the piece within its (t, k)
    jobs = []          # [t, base, piece_idx, tile_in_piece]
    idxcols = 0
    last_job_of_table = {}
    for t in range(NT):
        for k in range(NCH):
            ntk = int(T_tk[t, k])
            tile0 = 0
            while tile0 < ntk:
                nt_p = min(TP, ntk - tile0)
                p_idx = len(pieces)
                pieces.append((t, k, nt_p, idxcols))
                piece_tile0.append(tile0)
                for i in range(nt_p):
                    gt = tile0 + i
                    bags = []
                    for c in range(NCORE):
                        s = seg_bag[(c, t, k)][gt * TILE:(gt + 1) * TILE]
                        if len(s):
                            bags.append([int(s.min()), int(s.max())])
                    if not bags:
                        jobs.append([t, 0, p_idx, i])
                        last_job_of_table[t] = len(jobs) - 1
                        continue
                    lo = min(b[0] for b in bags)
                    hi = max(b[1] for b in bags)
                    # greedy: snapped windows covering [lo, hi]
                    cur = lo
                    while True:
                        b = _snap(min(cur, BC - W))
                        jobs.append([t, b, p_idx, i])
                        last_job_of_table[t] = len(jobs) - 1
                        if b + W > hi:
                            break
                        cur = b + W
                idxcols += nt_p * (TILE // 16)
                tile0 += nt_p
    njobs = len(jobs)

    # Z column starts in r_all: each table's pair-block starts on an even
    # column so the bf16 reduce output stays 4-byte aligned
    zoff = []
    o = 64
    for n in range(1, NT + 1):
        zoff.append(o)
        o += n + (n & 1)
    assert o <= 512, o

    idx16 = np.zeros((NCORE, 16, idxcols), np.int16)
    gcnt = np.zeros((NCORE, len(pieces)), np.int32)
    for c in range(NCORE):
        for p_idx, (t, k, nt_p, col0) in enumerate(pieces):
            tile0 = piece_tile0[p_idx]
            loc = seg_loc[(c, t, k)]
            n = len(loc)
            lo = tile0 * TILE
            hi = min(n, (tile0 + nt_p) * TILE)
            # trailing pads are -1: the ucode skips their descriptors when
            # num_idxs_reg carries this core's valid count
            chunk = np.full(nt_p * TILE, -1, np.int16)
            if hi > lo:
                chunk[: hi - lo] = loc[lo:hi]
            gcnt[c, p_idx] = max(0, hi - lo)
            # element i -> [i % 16, i // 16]
            idx16[c, :, col0:col0 + nt_p * (TILE // 16)] = chunk.reshape(-1, 16).T

    # balance gather pieces across the 4 SWDGE queues by tile count (LPT)
    qmap = [0] * len(pieces)
    qload = [0] * 4
    for p_idx in sorted(range(len(pieces)), key=lambda p: -pieces[p][2]):
        q = qload.index(min(qload))
        qmap[p_idx] = q
        qload[q] += pieces[p_idx][2]

    idx16_rep = np.tile(idx16, (1, 8, 1))      # replicate into 8 bands of 16
    static = dict(pieces=pieces, piece_tile0=piece_tile0, jobs=jobs,
                  idxcols=idxcols, njobs=njobs, T_tk=T_tk,
                  last_job_of_table=last_job_of_table,
                  seg_bag=seg_bag, zoff=zoff, qmap=qmap)
    return static, idx16_rep, gcnt


def _pack_a(static, f8_dtype):
    """Per-core one-hot A matrices [128 rows, njobs, 128 cols] in fp8.
    Row r of job (tile) maps to col (bag - base) when it falls in the job's
    window; rows claimed by an earlier job of the same tile are left zero."""
    pieces = static["pieces"]
    piece_tile0 = static["piece_tile0"]
    jobs = static["jobs"]
    njobs = static["njobs"]
    seg_bag = static["seg_bag"]

    a_all = np.zeros((NCORE, TILE, njobs, W), f8_dtype)
    one = np.ones((), f8_dtype)
    for c in range(NCORE):
        prev_tile = None
        claimed = None
        for j, (t, base, p_idx, i) in enumerate(jobs):
            _t, k, nt_p, _c0 = pieces[p_idx]
            gt = piece_tile0[p_idx] + i
            seg = seg_bag[(c, t, k)][gt * TILE:(gt + 1) * TILE]
            if not len(seg):
                continue
            if prev_tile != (p_idx, i):
                prev_tile = (p_idx, i)
                claimed = np.zeros(len(seg), bool)
            rel = seg - base
            m = (rel >= 0) & (rel < W) & ~claimed
            claimed |= m
            rows = np.nonzero(m)[0]
            a_all[c, rows, j, rel[m]] = one
    return a_all


# ---------------------------------------------------------------- program
def _build(static):
    import concourse.tile as tile
    from concourse import bacc, mybir
    from concourse.masks import make_identity

    f32 = mybir.dt.float32
    bf16 = mybir.dt.bfloat16
    fp16 = mybir.dt.float16
    f8 = mybir.dt.float8e4
    i16 = mybir.dt.int16
    i32 = mybir.dt.int32
    AF = mybir.ActivationFunctionType
    ALU = mybir.AluOpType
    AX = mybir.AxisListType

    pieces = static["pieces"]
    jobs = static["jobs"]
    idxcols = static["idxcols"]
    njobs = static["njobs"]
    last_job_of_table = static["last_job_of_table"]
    zoff = static["zoff"]
    qmap = static["qmap"]

    nc = bacc.Bacc("TRN2", target_bir_lowering=False, debug=False,
                   num_devices=NCORE, num_swdge_queues=4)

    tab = nc.dram_tensor("tab", [NT, V, D], f32, kind="ExternalInput").ap()
    idx16_d = nc.dram_tensor("idx16", [128, idxcols], i16, kind="ExternalInput").ap()
    aall_d = nc.dram_tensor("aall", [128, njobs, W], f8, kind="ExternalInput").ap()
    gcnt_d = nc.dram_tensor("gcnt", [1, len(pieces)], i32, kind="ExternalInput").ap()
    xT_d = nc.dram_tensor("xT", [13, BC], f32, kind="ExternalInput").ap()
    w1t_d = nc.dram_tensor("w1t", [13, 512], f32, kind="ExternalInput").ap()
    w2t_d = nc.dram_tensor("w2t", [128, 4, 256], f32, kind="ExternalInput").ap()
    w3t_d = nc.dram_tensor("w3t", [128, 2, 64], f32, kind="ExternalInput").ap()
    b1_d = nc.dram_tensor("b1", [128, 4], f32, kind="ExternalInput").ap()
    b2_d = nc.dram_tensor("b2", [128, 2], f32, kind="ExternalInput").ap()
    b3_d = nc.dram_tensor("b3", [64, 1], f32, kind="ExternalInput").ap()
    tw1t_d = nc.dram_tensor("tw1t", [128, 4, 512], fp16, kind="ExternalInput").ap()
    tw2t_d = nc.dram_tensor("tw2t", [128, 4, 256], fp16, kind="ExternalInput").ap()
    tw3t_d = nc.dram_tensor("tw3t", [128, 2, 1], fp16, kind="ExternalInput").ap()
    tb1_d = nc.dram_tensor("tb1", [128, 4], f32, kind="ExternalInput").ap()
    tb2_d = nc.dram_tensor("tb2", [128, 2], f32, kind="ExternalInput").ap()
    tb3_d = nc.dram_tensor("tb3", [1, 1], f32, kind="ExternalInput").ap()
    out_d = nc.dram_tensor("out", [BC, 1], f32, kind="ExternalOutput").ap()

    idx16_s = nc.alloc_sbuf_tensor("idx16_s", [128, idxcols], i16).ap()
    gcnt_s = nc.alloc_sbuf_tensor("gcnt_s", [1, len(pieces)], i32).ap()
    zed = nc.alloc_sbuf_tensor("zed", [64, 512], f32).ap()
    tall = nc.alloc_sbuf_tensor("tall", [128, NBLK, NT + 1, D], bf16).ap()
    r_all = nc.alloc_sbuf_tensor("r_all", [128, NBLK, 512], bf16).ap()
    itmp = nc.alloc_sbuf_tensor("itmp", [128, NT, D], bf16).ap()
    xT_s = nc.alloc_sbuf_tensor("xT_s", [13, BC], f32).ap()
    w1t_s = nc.alloc_sbuf_tensor("w1t_s", [13, 512], f32).ap()
    w2t_s = nc.alloc_sbuf_tensor("w2t_s", [128, 4, 256], f32).ap()
    w3t_s = nc.alloc_sbuf_tensor("w3t_s", [128, 2, 64], f32).ap()
    b1_s = nc.alloc_sbuf_tensor("b1_s", [128, 4], f32).ap()
    b2_s = nc.alloc_sbuf_tensor("b2_s", [128, 2], f32).ap()
    b3_s = nc.alloc_sbuf_tensor("b3_s", [64, 1], f32).ap()
    tw1t_s = nc.alloc_sbuf_tensor("tw1t_s", [128, 4, 512], fp16).ap()
    tw2t_s = nc.alloc_sbuf_tensor("tw2t_s", [128, 4, 256], fp16).ap()
    tw3t_s = nc.alloc_sbuf_tensor("tw3t_s", [128, 2, 1], fp16).ap()
    tb1_s = nc.alloc_sbuf_tensor("tb1_s", [128, 4], f32).ap()
    tb2_s = nc.alloc_sbuf_tensor("tb2_s", [128, 2], f32).ap()
    tb3_s = nc.alloc_sbuf_tensor("tb3_s", [1, 1], f32).ap()
    hb1 = nc.alloc_sbuf_tensor("hb1", [128, 4, BC], f32).ap()
    hb2 = nc.alloc_sbuf_tensor("hb2", [128, 2, BC], f32).ap()
    h3t = nc.alloc_sbuf_tensor("h3t", [64, BC], f32).ap()
    ht1 = nc.alloc_sbuf_tensor("ht1", [128, 4, BC], bf16).ap()
    ht2 = nc.alloc_sbuf_tensor("ht2", [128, 2, BC], bf16).ap()
    ident = nc.alloc_sbuf_tensor("ident", [128, 128], f32).ap()
    identb = nc.alloc_sbuf_tensor("identb", [128, 128], bf16).ap()
    p_s = nc.alloc_sbuf_tensor("p_s", [1, BC], f32).ap()

    with tile.TileContext(nc) as tc:
        with (
            tc.tile_pool(name="g", bufs=8) as gp,
            tc.tile_pool(name="ab", bufs=4) as abp,
            tc.tile_pool(name="gbf", bufs=6) as gbfp,
            tc.tile_pool(name="tb", bufs=2) as tbp,
            tc.tile_pool(name="pp", bufs=2, space="PSUM") as pp,
            tc.tile_pool(name="mp", bufs=1, space="PSUM") as mp,
            tc.tile_pool(name="tp", bufs=1, space="PSUM") as tpp,
            tc.tile_pool(name="tpb", bufs=2, space="PSUM") as tpbp,
        ):
            # ---- input loads
            nc.sync.dma_start(idx16_s, idx16_d)
            nc.sync.dma_start(gcnt_s, gcnt_d)
            nc.sync.dma_start(xT_s, xT_d)
            nc.vector.memset(zed, 0.0)
            for a, b_ in [(w1t_s, w1t_d), (w2t_s, w2t_d), (w3t_s, w3t_d),
                          (b1_s, b1_d), (b2_s, b2_d), (b3_s, b3_d),
                          (tw1t_s, tw1t_d), (tw2t_s, tw2t_d), (tw3t_s, tw3t_d),
                          (tb1_s, tb1_d), (tb2_s, tb2_d), (tb3_s, tb3_d)]:
                nc.sync.dma_start(a, b_)
            make_identity(nc, ident)
            nc.vector.tensor_copy(identb, ident)
            nc.vector.memset(r_all, 0.0)
            nc.vector.memset(tall, 0.0)

            # ---- bottom MLP (no dependence on gathers; overlaps them)
            for m in range(4):
                for n2 in range(2):
                    ps = mp.tile([128, 512], f32, tag="mp")
                    nc.tensor.matmul(ps, w1t_s[:, m * 128:(m + 1) * 128],
                                     xT_s[:, n2 * 512:(n2 + 1) * 512],
                                     start=True, stop=True)
                    nc.scalar.activation(hb1[:, m, n2 * 512:(n2 + 1) * 512], ps,
                                         AF.Relu, bias=b1_s[:, m:m + 1])
            for m in range(2):
                for n2 in range(2):
                    ps = mp.tile([128, 512], f32, tag="mp")
                    for kk in range(4):
                        nc.tensor.matmul(ps, w2t_s[:, kk, m * 128:(m + 1) * 128],
                                         hb1[:, kk, n2 * 512:(n2 + 1) * 512],
                                         start=(kk == 0), stop=(kk == 3))
                    nc.scalar.activation(hb2[:, m, n2 * 512:(n2 + 1) * 512], ps,
                                         AF.Relu, bias=b2_s[:, m:m + 1])
            for n2 in range(2):
                ps = mp.tile([128, 512], f32, tag="mp")
                for kk in range(2):
                    nc.tensor.matmul(ps[0:64, :], w3t_s[:, kk, :],
                                     hb2[:, kk, n2 * 512:(n2 + 1) * 512],
                                     start=(kk == 0), stop=(kk == 1))
                nc.scalar.activation(h3t[:, n2 * 512:(n2 + 1) * 512], ps[0:64, :],
                                     AF.Relu, bias=b3_s)
            for blk in range(NBLK):
                ps = tpp.tile([128, 128], f32, tag="tp")
                nc.tensor.transpose(ps[:, 0:64],
                                    h3t[:, blk * 128:(blk + 1) * 128],
                                    ident[0:64, 0:64])
                nc.vector.tensor_copy(r_all[:, blk, 0:64], ps[:, 0:64])
                nc.scalar.activation(tall[:, blk, 0, :], ps[:, 0:64], AF.Copy)

            # ---- gather + pool (transposed: gbf stationary, fp8 A moving)
            job_ptr = 0
            a_tile = None
            a_base = -1
            pooledT = None
            cur_t = -1

            def finish_table(t):
                # PSUM pooledT [64, BC] -> SBUF bf16 (bank-sized halves),
                # then per-block transpose to sample-major tall[:, blk, t+1, :]
                tT = tbp.tile([64, BC], bf16, tag="tb")
                nc.scalar.activation(tT[:, 0:BC // 2], pooledT[:, 0:BC // 2],
                                     AF.Copy)
                nc.scalar.activation(tT[:, BC // 2:], pooledT[:, BC // 2:],
                                     AF.Copy)
                for blk in range(NBLK):
                    tps = tpbp.tile([128, 128], bf16, tag="tpb")
                    nc.tensor.transpose(
                        tps[:, 0:64],
                        tT[:, blk * 128:(blk + 1) * 128],
                        identb[0:64, 0:64])
                    nc.scalar.activation(tall[:, blk, t + 1, :], tps[:, 0:64],
                                         AF.Copy)
                # emit table t+1's interaction terms now so the Vector queue
                # interleaves them with the remaining tables' gathers
                interaction_chunk(t + 1)

            def interaction_chunk(n):
                with nc.allow_low_precision(reason="64-elem bf16 dots into "
                                            "a bf16 top MLP; within rel tol"):
                    for blk in range(NBLK):
                        nc.vector.tensor_tensor(
                            itmp[:, 0:n, :],
                            tall[:, blk, n, None, :].to_broadcast([128, n, 64]),
                            tall[:, blk, 0:n, :],
                            op=ALU.mult,
                        )
                        nc.vector.tensor_reduce(
                            r_all[:, blk, zoff[n - 1]:zoff[n - 1] + n],
                            itmp[:, 0:n, :], axis=AX.X, op=ALU.add,
                        )

            creg_cm = nc.gpsimd.register("gcnt_reg")
            creg = creg_cm.__enter__()
            for p_idx, (t, k, nt_p, col0) in enumerate(pieces):
                if t != cur_t:
                    if pooledT is not None:
                        finish_table(cur_t)
                    pooledT = pp.tile([64, BC], f32, tag="pp")
                    # zero (bank-sized halves) on the Scalar engine; the
                    # pool matmuls then accumulate with start=False
                    nc.scalar.activation(pooledT[:, 0:BC // 2],
                                         zed, AF.Copy)
                    nc.scalar.activation(pooledT[:, BC // 2:],
                                         zed, AF.Copy)
                    cur_t = t
                rows = min(V - k * SCH, SCH)
                src = tab[t, k * SCH:k * SCH + rows, :]
                g = gp.tile([128, TP, 64], f32, tag="gbuf")
                n_idx = nt_p * TILE
                nc.gpsimd.reg_load(creg, gcnt_s[0:1, p_idx:p_idx + 1])
                nc.gpsimd.dma_gather(
                    g[:, :nt_p, :], src,
                    idx16_s[:, col0:col0 + nt_p * 8],
                    num_idxs=n_idx, num_idxs_reg=creg,
                    elem_size=64, elem_step=64, single_packet=False,
                    queue_num=qmap[p_idx],
                )
                gbf = gbfp.tile([128, TP, 64], bf16, tag="gbf")
                nc.scalar.activation(gbf[:, :nt_p, :], g[:, :nt_p, :], AF.Copy)
                while job_ptr < len(jobs) and jobs[job_ptr][2] == p_idx:
                    jt, base, _p, i = jobs[job_ptr]
                    if a_tile is None or job_ptr >= a_base + JB:
                        a_base = job_ptr
                        nb = min(JB, njobs - a_base)
                        a_tile = abp.tile([128, JB, W], f8, tag="ab")
                        nc.sync.dma_start(a_tile[:, :nb, :],
                                          aall_d[:, a_base:a_base + nb, :])
                    nc.tensor.matmul(pooledT[:, base:base + W],
                                     gbf[:, i, :],
                                     a_tile[:, job_ptr - a_base, :],
                                     start=False,
                                     stop=(job_ptr == last_job_of_table[jt]),
                                     skip_group_check=True)
                    job_ptr += 1
            assert job_ptr == len(jobs)
            creg_cm.__exit__(None, None, None)
            if pooledT is not None:
                finish_table(cur_t)

            # ---- top MLP (R^T tiles share the gather pool slots)
            rt = []
            for f in range(4):
                rt.append(gp.tile([128, 1024], bf16, tag="gbuf", name=f"rt{f}"))
            nc.vector.memset(rt[3][:, :], 0.0)
            for f in range(4):
                wdt = 128
                for blk in range(NBLK):
                    ps = tpbp.tile([128, 128], bf16, tag="tpb")
                    nc.tensor.transpose(ps[0:wdt, :],
                                        r_all[:, blk, f * 128:f * 128 + wdt],
                                        identb)
                    nc.vector.tensor_copy(
                        rt[f][0:wdt, blk * 128:(blk + 1) * 128], ps[0:wdt, :])
            for m in range(4):
                for n2 in range(2):
                    ps = mp.tile([128, 512], f32, tag="mp")
                    for kk in range(4):
                        nc.tensor.matmul(ps, tw1t_s[:, kk, m * 128:(m + 1) * 128],
                                         rt[kk][:, n2 * 512:(n2 + 1) * 512],
                                         start=(kk == 0), stop=(kk == 3))
                    nc.scalar.activation(ht1[:, m, n2 * 512:(n2 + 1) * 512], ps,
                                         AF.Relu, bias=tb1_s[:, m:m + 1])
            for m in range(2):
                for n2 in range(2):
                    ps = mp.tile([128, 512], f32, tag="mp")
                    for kk in range(4):
                        nc.tensor.matmul(ps, tw2t_s[:, kk, m * 128:(m + 1) * 128],
                                         ht1[:, kk, n2 * 512:(n2 + 1) * 512],
                                         start=(kk == 0), stop=(kk == 3))
                    nc.scalar.activation(ht2[:, m, n2 * 512:(n2 + 1) * 512], ps,
                                         AF.Relu, bias=tb2_s[:, m:m + 1])
            for n2 in range(2):
                ps = mp.tile([128, 512], f32, tag="mp")
                for kk in range(2):
                    nc.tensor.matmul(ps[0:1, :], tw3t_s[:, kk, :],
                                     ht2[:, kk, n2 * 512:(n2 + 1) * 512],
                                     start=(kk == 0), stop=(kk == 1))
                nc.scalar.activation(p_s[:, n2 * 512:(n2 + 1) * 512], ps[0:1, :],
                                     AF.Sigmoid, bias=tb3_s)
            nc.sync.dma_start(out_d.rearrange("a b -> b a"), p_s)

    nc.compile()
    return nc


# ---------------------------------------------------------------- ntff shim
def _install_ntff_shim():
    """Provide antenv.axon_hooks so run_bass_kernel_spmd(trace=True) can pull
    NTFF profiles through libaxon_pjrt (module is absent in this image)."""
    import types
    if "antenv.axon_hooks" in sys.modules:
        return
    try:
        import antenv
        from trn_agent_boot.trn_boot import _ntff_profile_via_ctypes
    except Exception:
        return
    mod = types.ModuleType("antenv.axon_hooks")
    _state = {"hook": None}
    mod.set_axon_ntff_profile_hook = lambda h: _state.__setitem__("hook", h)
    mod.get_axon_ntff_profile_hook = lambda: _state["hook"]
    sys.modules["antenv.axon_hooks"] = mod
    antenv.axon_hooks = mod
    try:
        hook = _ntff_profile_via_ctypes("/opt/axon/libaxon_pjrt.so")
        mod.set_axon_ntff_profile_hook(hook)
    except Exception:
        pass


# ---------------------------------------------------------------- entry
def kernel(x, lS_i, lS_o, emb_tables,
           bot_W1, bot_b1, bot_W2, bot_b2, bot_W3, bot_b3,
           top_W1, top_b1, top_W2, top_b2, top_W3, top_b3):
    from concourse import mybir

    t0 = time.time()
    static, idx16_rep, gcnt = _prep(lS_i)
    a_all = _pack_a(static, mybir.dt.np(mybir.dt.float8e4))
    t1 = time.time()

    nc = _build(static)
    t2 = time.time()

    tabf = np.ascontiguousarray(np.asarray(emb_tables, dtype=np.float32))
    xT = np.ascontiguousarray(np.asarray(x, np.float32).T)          # [13, B]
    w1t = np.ascontiguousarray(np.asarray(bot_W1, np.float32).T)    # [13, 512]
    w2t = np.ascontiguousarray(
        np.asarray(bot_W2, np.float32).T.reshape(4, 128, 256).transpose(1, 0, 2))
    w3t = np.ascontiguousarray(
        np.asarray(bot_W3, np.float32).T.reshape(2, 128, 64).transpose(1, 0, 2))
    b1 = np.ascontiguousarray(np.asarray(bot_b1, np.float32).reshape(4, 128).T)
    b2 = np.ascontiguousarray(np.asarray(bot_b2, np.float32).reshape(2, 128).T)
    b3 = np.asarray(bot_b3, np.float32).reshape(64, 1)
    tw1 = np.zeros((512, 512), np.float32)
    w1T = np.asarray(top_W1, np.float32).T    # [415, 512]
    tw1[0:64, :] = w1T[0:64, :]
    zoff = static["zoff"]
    off = 64
    for n in range(1, NT + 1):
        tw1[zoff[n - 1]:zoff[n - 1] + n, :] = w1T[off:off + n, :]
        off += n
    tw1t = np.ascontiguousarray(
        tw1.reshape(4, 128, 512).transpose(1, 0, 2)).astype(np.float16)
    tw2t = np.ascontiguousarray(
        np.asarray(top_W2, np.float32).T.reshape(4, 128, 256)
        .transpose(1, 0, 2)).astype(np.float16)
    tw3t = np.ascontiguousarray(
        np.asarray(top_W3, np.float32).T.reshape(2, 128, 1)
        .transpose(1, 0, 2)).astype(np.float16)
    tb1 = np.ascontiguousarray(np.asarray(top_b1, np.float32).reshape(4, 128).T)
    tb2 = np.ascontiguousarray(np.asarray(top_b2, np.float32).reshape(2, 128).T)
    tb3 = np.asarray(top_b3, np.float32).reshape(1, 1)

    in_maps = []
    for c in range(NCORE):
        in_maps.append({
            "tab": tabf,
            "idx16": np.ascontiguousarray(idx16_rep[c]),
            "aall": np.ascontiguousarray(a_all[c]),
            "gcnt": np.ascontiguousarray(gcnt[c:c + 1]),
            "xT": np.ascontiguousarray(xT[:, c * BC:(c + 1) * BC]),
            "w1t": w1t, "w2t": w2t, "w3t": w3t,
            "b1": b1, "b2": b2, "b3": b3,
            "tw1t": tw1t, "tw2t": tw2t, "tw3t": tw3t,
            "tb1": tb1, "tb2": tb2, "tb3": tb3,
        })

    from concourse.bass_utils import run_bass_kernel_spmd
    do_trace = bool(os.environ.get("DLRM_TRACE"))
    if do_trace:
        _install_ntff_shim()
    res = run_bass_kernel_spmd(nc, in_maps, core_ids=list(range(NCORE)),
                               trace=do_trace)
    t3 = time.time()
    if do_trace:
        with open("/tmp/dlrm_exec_ns.txt", "w") as f:
            f.write(str(res.exec_time_ns))
    print(f"[kernel] prep {t1-t0:.1f}s build+compile {t2-t1:.1f}s "
          f"run {t3-t2:.1f}s exec_ns={res.exec_time_ns}", file=sys.stderr)
    out = np.concatenate([r["out"] for r in res.results], axis=0)
    return out.astype(np.float32)
